# revision 65
# baseline (speedup 1.0000x reference)
"""Multi-head causal attention (B=4, T=2048, C=1024, H=16, D=64) on 8 trn2 cores.

Sharding: core c owns batch b = c//2 and heads g*8..g*8+7 where g = c%2
(batch-parallel x head-tensor-parallel). Each core computes its 8 heads'
QKV projections, causal attention, and a partial output projection
(columns of Wp belonging to its heads). Host sums the two head-group
partials per batch and adds the bias.

Dtypes (PSUM accumulation always f32):
 - QKV projections: fp8e4m3 value+residual splits of x and 64*W, prepared
   on the host. Three DoubleRow chains (x8*W8 + x8*Wr + xr8*W8) give
   near-bf16 accuracy at 0.5 cycles/row - 25% fewer PE cycles than bf16
   and 2x fewer than f32.
 - scores: fp8e4m3 Q/K with DoubleRow. Q/K are evacuated from the
   projection PSUM straight into tiles laid out [32 partitions, d-half, T]
   per head: the host orders Wq/Wk columns so head a's d-low rows land on
   partitions 32a..32a+31, making the evacuation partition-preserving.
   Matmul APs may only start at partitions 0/32/64, so the a=3 heads
   (partitions 96..127) get a small DMA shift into base-0 copies.
 - everything else (exp strips, V, AV, Y, Wp, o): bf16.
Measured end-to-end error vs the f32 reference ~8.6e-3 (tolerance 2e-2).

Scheduling: one global PE instruction stream, ordered so the ACT engine
(exp - the #2 engine at ~165us busy vs PE ~186us) starts by ~12us and is
never starved mid-stream. Projection and output-projection work is chopped
into ~1-2us units and woven between attention strip-passes as PE filler.
AV matmuls trail scores/exp by LAG strips (software pipeline); per-chunk
softmax normalization (1/rowsum broadcast via a rank-1 PE outer product)
runs as soon as a chunk's last strip lands. Normalized head outputs stay
in SBUF: even heads' rows land on partitions 0..63 directly; odd heads are
moved to partitions 64..127 with a matmul against a shifted identity
(DVE cannot cross partitions). The output projection reads Y from SBUF,
is woven into the second half-passes per chunk, and DMAs out o^T slices
as they complete.

Matmul shapes (out = lhsT.T @ rhs, contraction on partitions):
  QT/KT:  lhsT = W8[ckpair, 2, m-tile] rhs = x8[ckpair, 2, t-chunk] (DR)
  V:      lhsT = x8[ckpair, 2, s-tile] rhs = Wv8[ckpair, 2, :]      (DR)
  scoresT: lhsT = K2[32, 2, s-tile]    rhs = Q2[32, 2, t-seg]       (DR)
  AV^T:   lhsT = [V_h | 1][s-tile,65]  rhs = expT strip [s-tile, t]
  out^T:  lhsT = WpS[j-tile, c-tile]   rhs = Y[j-tile, t-chunk]
"""

import numpy as np
import ml_dtypes
from contextlib import ExitStack

B, T, C, H, D = 4, 2048, 1024, 16, 64
HL = H // 2          # 8 heads per core
N_CORES = 8
P = 128
NK = C // P          # 8 contraction tiles for projections
NM = HL * D // P     # 4 m-tiles of Q/K head-dims
NS = T // P          # 16 s-tiles (key strips)
CH = 512             # t-chunk width
NCH = T // CH        # 4 t-chunks
LAG = 14             # AV trails scores by this many strips

_nc_cache = None


def build_nc():
    global _nc_cache
    if _nc_cache is not None:
        return _nc_cache
    import concourse.bass as bass  # noqa: F401
    import concourse.tile as tile
    from concourse import bacc, mybir

    f32 = mybir.dt.float32
    bf16 = mybir.dt.bfloat16
    fp8 = mybir.dt.float8e4
    Exp = mybir.ActivationFunctionType.Exp
    DR = mybir.MatmulPerfMode.DoubleRow

    def mm(out, lhsT, rhs, **kw):
        nc.tensor.matmul(out, lhsT=lhsT, rhs=rhs, **kw)

    nc = bacc.Bacc("TRN2", target_bir_lowering=False, debug=False,
                   enable_asserts=True, num_devices=N_CORES)
    # x and the QKV weights ship as fp8 value+residual pairs, partition-
    # major: x as [P, NK/2, 2(k-pair), T], W as [P, NK/2, 2, 512], weights
    # prescaled by 64 (psum then holds 64*Q etc.)
    NKP = NK // 2
    x8 = nc.dram_tensor("x8", (P, NKP, 2, T), fp8, kind="ExternalInput").ap()
    xr8 = nc.dram_tensor("xr8", (P, NKP, 2, T), fp8, kind="ExternalInput").ap()
    w8 = [nc.dram_tensor(f"w8_{i}", (P, NKP, 2, HL * D), fp8,
                         kind="ExternalInput").ap() for i in range(3)]
    wr8 = [nc.dram_tensor(f"wr8_{i}", (P, NKP, 2, HL * D), fp8,
                          kind="ExternalInput").ap() for i in range(3)]
    wps = nc.dram_tensor("wps", (HL * D, C), bf16, kind="ExternalInput").ap()
    tri = nc.dram_tensor("tri", (P, 2 * P), bf16, kind="ExternalInput").ap()
    ones = nc.dram_tensor("ones", (P, 2 * P), bf16, kind="ExternalInput").ap()
    # shifti[k, 64+k] = 1: a matmul against it moves rows down 64 partitions
    shifti = nc.dram_tensor("shifti", (D, P), bf16, kind="ExternalInput").ap()
    o = nc.dram_tensor("o", (C, T), bf16, kind="ExternalOutput").ap()

    with tile.TileContext(nc) as tc:
        with ExitStack() as ctx:
            ctx.enter_context(nc.allow_low_precision(
                reason="bf16/fp8 operands with f32 PSUM accumulation"))
            # PSUM banks (8 total): scores 2x[128,1024]=4, small-mm
            # (proj/outproj/rank1/shift) 2x[128,512]=2, av 2x[65,512]=2.
            # Separate tags so filler units never block the scores->exp
            # pipeline's psum rotation.
            sc_ps = ctx.enter_context(tc.tile_pool(name="sc_ps", bufs=2, space="PSUM"))
            sm_ps = ctx.enter_context(tc.tile_pool(name="sm_ps", bufs=2, space="PSUM"))
            av_ps = ctx.enter_context(tc.tile_pool(name="av_ps", bufs=2, space="PSUM"))

            const_pool = ctx.enter_context(tc.tile_pool(name="const", bufs=1))
            tri_sb = const_pool.tile([P, 2 * P], bf16, name="tri_sb", tag="tri_sb")
            ones_sb = const_pool.tile([P, D], bf16, name="ones_sb", tag="ones_sb")
            shifti_sb = const_pool.tile([D, P], bf16, name="shifti_sb",
                                        tag="shifti_sb")

            main = ctx.enter_context(tc.tile_pool(name="main", bufs=1))
            # Q2/K2: [partition 32a+dl, g, u(d-half), t] fp8 - head h=4g+a's
            # (d = 32u+dl) value lives at partition 32a+dl, free (g, u, t).
            # Matmul APs may only start at partitions 0/32/64, so the a=3
            # heads (partitions 96..127) are DMA-shifted into base-0 copies.
            Q2 = main.tile([P, 2, 2, T], fp8, name="q2", tag="q2")
            K2 = main.tile([P, 2, 2, T], fp8, name="k2", tag="k2")
            Q2x = main.tile([32, 2, 2, T], fp8, name="q2x", tag="q2x")
            K2x = main.tile([32, 2, 2, T], fp8, name="k2x", tag="k2x")
            # V: [s-within-tile, s-tile, head, d+1]; col 64 = ones (rowsum)
            Vsb = main.tile([P, NS, HL, D + 1], bf16, name="vsb", tag="vsb")
            Wp_sb = [main.tile([P, C], bf16, name=f"wp{j}", tag=f"wp{j}")
                     for j in range(NM)]
            # normalized head outputs YT, SBUF-resident: tile m holds heads
            # 2m (partitions 0..63) and 2m+1 (64..127)
            Ysb = [main.tile([P, T], bf16, name=f"y{m}", tag=f"y{m}")
                   for m in range(NM)]

            wpool = ctx.enter_context(tc.tile_pool(name="wpool", bufs=1))
            # one consolidated DMA per weight tensor: [p, k-pair, 2, m]
            W_sb = [wpool.tile([P, NKP, 2, HL * D], fp8, name=f"w8{pj}",
                               tag=f"w8{pj}") for pj in range(3)]
            Wr_sb = [wpool.tile([P, NKP, 2, HL * D], fp8, name=f"wr{pj}",
                                tag=f"wr{pj}") for pj in range(3)]

            def dma_w(pj, eng):
                eng.dma_start(out=W_sb[pj], in_=w8[pj])
                eng.dma_start(out=Wr_sb[pj], in_=wr8[pj])

            xpool = ctx.enter_context(tc.tile_pool(name="xpool", bufs=2))
            xs_tiles = {}

            def dma_x(ch, eng=None, split=False):
                xs = xpool.tile([P, NKP, 2, CH], fp8, name="xs", tag="xs")
                xr = xpool.tile([P, NKP, 2, CH], fp8, name="xr", tag="xr")
                e = eng or nc.scalar
                e.dma_start(out=xs, in_=x8[:, :, :, ch * CH:(ch + 1) * CH])
                if not split:
                    e.dma_start(out=xr, in_=xr8[:, :, :, ch * CH:(ch + 1) * CH])
                    xs_tiles[ch] = (xs, xr)
                    return
                xs_tiles[ch] = (xs, xr)

                def later():
                    e.dma_start(out=xr, in_=xr8[:, :, :, ch * CH:(ch + 1) * CH])
                return later

            strip_pool = ctx.enter_context(tc.tile_pool(name="strip_pool", bufs=18))
            small = ctx.enter_context(tc.tile_pool(name="small", bufs=4))
            tmp_pool = ctx.enter_context(tc.tile_pool(name="tmp_pool", bufs=4))
            obpool = ctx.enter_context(tc.tile_pool(name="obpool", bufs=6))

            # ---------------- work units ----------------
            def pu_qk(proj, m, ch):
                """Q or K projection unit -> fp8 split tiles. m = 2g+u.
                Split-fp8: psum = x8*W8 + x8*Wr + xr8*W8 ~= 64*Q exactly."""
                def run():
                    xs, xr = xs_tiles[ch]
                    dst = Q2 if proj == 0 else K2
                    g, u = m // 2, m % 2
                    ms = slice(m * P, (m + 1) * P)
                    ps = sm_ps.tile([P, CH], f32, name="qk_ps", tag="sm")
                    first = True
                    for wt, xt in ((W_sb[proj], xs), (Wr_sb[proj], xs),
                                   (W_sb[proj], xr)):
                        for kp in range(NKP):
                            mm(ps, wt[:, kp, :, ms], xt[:, kp, :, :],
                               perf_mode=DR, start=first,
                               stop=(not first and wt is W_sb[proj]
                                     and xt is xr and kp == NKP - 1))
                            first = False
                    dslice = dst[:, g, u, ch * CH:(ch + 1) * CH]
                    if ch == 2:
                        nc.vector.tensor_copy(dslice, ps)
                    else:
                        nc.scalar.copy(dslice, ps)
                return run

            def pu_remap(proj, g, ch):
                """Shift head a=3's (g, ch) slice down to the base-0 copy."""
                def run():
                    src, dst = (Q2, Q2x) if proj == 0 else (K2, K2x)
                    nc.sync.dma_start(
                        out=dst[:, g, :, ch * CH:(ch + 1) * CH],
                        in_=src[96:128, g, :, ch * CH:(ch + 1) * CH])
                return run

            def pu_v(s):
                """V projection unit for s-tile s (psum = 64*V; evac /64)."""
                def run():
                    ch, sl = s // (CH // P), s % (CH // P)
                    xs, xr = xs_tiles[ch]
                    ss = slice(sl * P, (sl + 1) * P)
                    ps = sm_ps.tile([P, HL * D], f32, name="v_ps", tag="sm")
                    first = True
                    chains = ((xs, W_sb[2]), (xs, Wr_sb[2]), (xr, W_sb[2]))
                    for ci, (xt, wt) in enumerate(chains):
                        for kp in range(NKP):
                            mm(ps, xt[:, kp, :, ss], wt[:, kp, :, :],
                               perf_mode=DR, start=first,
                               stop=(ci == 2 and kp == NKP - 1))
                            first = False
                    # evac on ACT where it idles (pre-pass region);
                    # on DVE in G1 where ACT is the exp bottleneck. The
                    # 1/64 de-scale fuses into either copy.
                    if s < 8:
                        nc.scalar.activation(
                            Vsb[:, s, :, 0:D],
                            ps.rearrange("p (h d) -> p h d", h=HL),
                            mybir.ActivationFunctionType.Copy, scale=1.0 / 64)
                    else:
                        nc.vector.tensor_scalar_mul(
                            Vsb[:, s, :, 0:D],
                            ps.rearrange("p (h d) -> p h d", h=HL), 1.0 / 64)
                return run

            def op_unit(ct, ch):
                """Output-projection unit: o^T[ct-tile, ch-chunk]."""
                def run():
                    ps = sm_ps.tile([P, CH], f32, name="p_ps", tag="sm")
                    for j in range(NM):
                        mm(ps, Wp_sb[j][:, ct * P:(ct + 1) * P],
                           Ysb[j][:, ch * CH:(ch + 1) * CH],
                           start=(j == 0), stop=(j == NM - 1))
                    ob = obpool.tile([P, CH], bf16, name="ob", tag="ob")
                    if ch >= 2:
                        # tail chunks run after the last exp - ACT is free
                        nc.scalar.copy(ob, ps)
                    else:
                        nc.vector.tensor_copy(ob, ps)
                    eng = (nc.sync, nc.gpsimd)[(ct + ch) % 2]
                    eng.dma_start(
                        out=o[ct * P:(ct + 1) * P, ch * CH:(ch + 1) * CH], in_=ob)
                return run

            # ---------------- attention passes ----------------
            def make_pass(h, half):
                g, a = h // 4, h % 4
                tlo = half * 1024
                ns = 8 if half == 0 else NS
                st = {"strips": [None] * ns, "avs": None}

                if a < 3:
                    qt, kt, pbase = Q2, K2, 32 * a
                else:
                    qt, kt, pbase = Q2x, K2x, 0

                def do_scores(i):
                    t0 = P * i
                    s0 = max(t0, tlo)
                    strip = strip_pool.tile([P, 1024], bf16,
                                            name="strip", tag="strip")
                    st["strips"][i] = strip
                    ps = sc_ps.tile([P, 1024], f32, name="sc_ps", tag="sc")
                    b0 = s0
                    while b0 < tlo + 1024:
                        b1 = min((b0 // CH + 1) * CH, tlo + 1024)
                        mm(ps[:, b0 - tlo:b1 - tlo],
                           kt[pbase:pbase + 32, g, :, t0:t0 + P],
                           qt[pbase:pbase + 32, g, :, b0:b1],
                           perf_mode=DR, start=True, stop=True)
                        b0 = b1
                    nc.scalar.activation(
                        strip[:, s0 - tlo:1024], ps[:, s0 - tlo:1024],
                        Exp, scale=float(1.0 / np.sqrt(D) / 4096))

                def do_av(i):
                    if st["avs"] is None:
                        st["avs"] = {j: av_ps.tile([D + 1, CH], f32,
                                                   name=f"av{j}", tag="av")
                                     for j in (2 * half, 2 * half + 1)}
                    avs = st["avs"]
                    t0 = P * i
                    strip = st["strips"][i]
                    if t0 >= tlo:              # diagonal block: causal mask
                        nc.vector.tensor_mul(
                            strip[:, t0 - tlo:t0 + P - tlo],
                            strip[:, t0 - tlo:t0 + P - tlo],
                            tri_sb[:, P:2 * P])
                    for j in (2 * half, 2 * half + 1):
                        if CH * (j + 1) <= t0:
                            continue
                        ts0 = max(CH * j, t0)
                        mm(avs[j][:, ts0 - CH * j:CH],
                           Vsb[:, i, h, :],
                           strip[:, ts0 - tlo:CH * (j + 1) - tlo],
                           start=(i == 0), stop=(i == 4 * j + 3),
                           skip_group_check=True)
                    if i % 4 == 3 and i // 4 in avs:
                        j = i // 4
                        rec = small.tile([D + 1, CH], bf16, name="rec", tag="rec")
                        nc.vector.reciprocal(rec[D:D + 1, :], avs[j][D:D + 1, :])
                        rps = sm_ps.tile([P, CH], f32, name="rps", tag="sm")
                        mm(rps[0:D, :], ones_sb[D:D + 1, 0:D], rec[D:D + 1, :],
                           start=True, stop=True)
                        # DVE reads only one PSUM operand; stage the broadcast
                        rsb = small.tile([D, CH], f32, name="rsb", tag="rsb")
                        nc.vector.tensor_copy(rsb, rps[0:D, :])
                        if h % 2 == 0:
                            # even head: rows land at partitions 0..63 directly
                            nc.vector.tensor_mul(
                                Ysb[h // 2][0:D, CH * j:CH * (j + 1)],
                                avs[j][0:D, :], rsb)
                        else:
                            # odd head: normalize into tmp, then a matmul
                            # against shifti moves the rows to partitions
                            # 64..127 (DVE cannot cross partitions)
                            tmpc = tmp_pool.tile([D, CH], bf16,
                                                 name="tmpc", tag="tmpc")
                            nc.vector.tensor_mul(tmpc, avs[j][0:D, :], rsb)
                            shp = sm_ps.tile([P, CH], f32, name="shp", tag="sm")
                            mm(shp, shifti_sb, tmpc, start=True, stop=True)
                            nc.vector.tensor_copy(
                                Ysb[h // 2][D:P, CH * j:CH * (j + 1)],
                                shp[D:P, :])

                return ([lambda i=i: do_scores(i) for i in range(ns)],
                        [lambda i=i: do_av(i) for i in range(ns)])

            # ---------------- global schedule ----------------
            # entries: ("s", scores_fn) participates in the LAG pipeline;
            #          ("f", filler_fn) is PE filler / DMA issue.
            entries = []
            avq = []

            def weave(pass_list, fillers, per, start_after=0):
                """Interleave `fillers` into the passes, `per` fillers after
                each scores entry (fractional via accumulator). The first
                filler is held back until `start_after` scores entries."""
                fi = -per * start_after
                fidx = 0
                for sc, ac in pass_list:
                    for s_fn, a_fn in zip(sc, ac):
                        entries.append(("s", s_fn))
                        avq.append(a_fn)
                        fi += per
                        while fidx + 1 <= fi and fidx < len(fillers):
                            entries.append(("f", fillers[fidx]))
                            fidx += 1
                while fidx < len(fillers):
                    entries.append(("f", fillers[fidx]))
                    fidx += 1

            # setup DMAs in priority order, spread across engine DGE queues:
            # the first projection units need x ch0 + wq/wk only.
            xr0_later = dma_x(0, nc.scalar, split=True)
            dma_w(0, nc.sync)
            xr0_later()
            dma_w(1, nc.gpsimd)
            dma_x(1, nc.sync)
            dma_w(2, nc.scalar)           # wv: needed from V s0 (~unit 9)
            nc.sync.dma_start(
                out=Vsb[:, :, :, D],
                in_=ones[:, 0:NS * HL].rearrange("p (s h) -> p s h", s=NS))
            nc.scalar.dma_start(out=tri_sb, in_=tri)
            nc.scalar.dma_start(out=ones_sb, in_=ones[:, 0:D])
            nc.gpsimd.dma_start(out=shifti_sb, in_=shifti)
            for j in range(NM):
                nc.gpsimd.dma_start(out=Wp_sb[j], in_=wps[j * P:(j + 1) * P, :])

            # minimal prologue: Q/K m0/m1 (heads 0-3) for chunks 0,1 plus the
            # first two V tiles; everything else weaves into the passes.
            pro = []
            for ch in (0, 1):
                for m in (0, 1):
                    pro.append(pu_qk(0, m, ch))
                    pro.append(pu_qk(1, m, ch))
            for ch in (0, 1):
                pro.append(pu_remap(0, 0, ch))
                pro.append(pu_remap(1, 0, ch))
            pro += [pu_v(0), pu_v(1)]
            for f in pro:
                entries.append(("f", f))

            passes0 = [make_pass(h, 0) for h in range(HL)]
            g1_order = [1, 3, 5, 7, 0, 2, 4, 6]
            passes1 = [make_pass(h, 1) for h in g1_order]

            # G0a: first half-0 pass, densely packed with the remaining V
            # tiles its AVs need (V s-tile i is read at stream index i+LAG)
            weave(passes0[:1], [pu_v(s) for s in range(2, 8)], per=6 / 8.0)

            # G0b: remaining half-0 passes; fillers = Q/K m2/m3 (heads 4-7,
            # needed by pass h4 at scores index 24 of this group), chunk-2
            # projections, x2/x3 DMAs
            g0_fill = []
            for m in (2, 3):
                for ch in (0, 1):
                    g0_fill.append(pu_qk(0, m, ch))
                    g0_fill.append(pu_qk(1, m, ch))
            for ch in (0, 1):
                g0_fill.append(pu_remap(0, 1, ch))
                g0_fill.append(pu_remap(1, 1, ch))
            g0_fill.append(lambda: dma_x(2))
            for m in range(NM):
                g0_fill.append(pu_qk(0, m, 2))
                g0_fill.append(pu_qk(1, m, 2))
                if m % 2 == 1:
                    g0_fill.append(pu_remap(0, m // 2, 2))
                    g0_fill.append(pu_remap(1, m // 2, 2))
            g0_fill.append(lambda: dma_x(3))
            # chunk-3 Q/K must complete before any half-1 scores (their
            # first segments read ch3 columns); weaving them as the final
            # G0b fillers preserves that while keeping ACT fed
            for m in range(NM):
                g0_fill.append(pu_qk(0, m, 3))
                g0_fill.append(pu_qk(1, m, 3))
                if m % 2 == 1:
                    g0_fill.append(pu_remap(0, m // 2, 3))
                    g0_fill.append(pu_remap(1, m // 2, 3))
            weave(passes0[1:], g0_fill, per=len(g0_fill) / 56.0)
            # G1a: first half-1 pass densely packed with V s8..15 (V
            # s-tile i is first read by AV(i) at stream index i+LAG, so a
            # 1-per-scores weave lands each tile well before its reader)
            weave(passes1[:1], [pu_v(s) for s in range(8, 16)], per=1.0)

            # G1b: remaining half-1 passes; fillers = chunk-0/1 output
            # projection (ready once all half-0 passes have fully drained
            # through the LAG pipeline - hold fillers back past that point)
            g1_fill = [op_unit(ct, 0) for ct in range(NK)]
            g1_fill += [op_unit(ct, 1) for ct in range(NK)]
            weave(passes1[1:], g1_fill, per=len(g1_fill) / 64.0,
                  start_after=2)

            # tail: chunk-2 output projection can start as soon as the last
            # pass's strip-11 AV (chunk-2 normalize, avq index 187) drains;
            # chunk-3 needs the full drain.
            inject = {64 + 7 * 16 + 11: [op_unit(ct, 2) for ct in range(NK)]}
            tail = [op_unit(ct, 3) for ct in range(NK)]

            # ---------------- execute the stream ----------------
            si = 0
            for kind, fn in entries:
                if kind == "f":
                    fn()
                    continue
                fn()
                if si >= LAG:
                    avq[si - LAG]()
                si += 1
            for k in range(max(si - LAG, 0), si):
                avq[k]()
                for fn in inject.get(k, ()):
                    fn()
            for fn in tail:
                fn()

    nc.compile()
    _nc_cache = nc
    return nc


def _split_qk(Warr):
    """[8, C, 64] -> [C, 512] with column m' = 128*(2g+u) + 32a + dl for
    head h=4g+a, d=32u+dl (matches the Q2/K2 on-device layout)."""
    arr = Warr.reshape(2, 4, C, 2, 32)        # [g, a, c, u, dl]
    arr = arr.transpose(2, 0, 3, 1, 4)        # [c, g, u, a, dl]
    return np.ascontiguousarray(arr.reshape(C, HL * D))


F8 = ml_dtypes.float8_e4m3


def _q8(v):
    return np.asarray(v, F8)


def _pair_layout(arr):
    """[C, m] -> [P, NK/2, 2, m] (contraction split into DoubleRow pairs)."""
    m = arr.shape[1]
    return np.ascontiguousarray(
        arr.reshape(NK // 2, 2, P, m).transpose(2, 0, 1, 3))


def _split8(arr):
    """fp8 value+residual pair of [C, m] array, in pair layout."""
    a8 = _q8(arr)
    r8 = _q8(arr - a8.astype(np.float32))
    return _pair_layout(a8), _pair_layout(r8)


def make_in_maps(x, Wq, Wk, Wv, Wp):
    """Shard FULL inputs into per-core input maps."""
    bf = ml_dtypes.bfloat16
    tri = np.concatenate(
        [np.zeros((P, P), dtype=np.float32),
         np.triu(np.ones((P, P), dtype=np.float32))], axis=1).astype(bf)
    ones = np.ones((P, 2 * P), dtype=np.float32).astype(bf)
    shifti = np.zeros((D, P), dtype=np.float32)
    shifti[np.arange(D), D + np.arange(D)] = 1.0
    shifti = shifti.astype(bf)
    in_maps = []
    for c in range(N_CORES):
        b, g = c // 2, c % 2
        hs = slice(g * HL, (g + 1) * HL)
        x8, xr8 = _split8(np.ascontiguousarray(x[b].T))
        ws = [_split_qk(Wq[hs]), _split_qk(Wk[hs]),
              np.ascontiguousarray(
                  Wv[hs].transpose(1, 0, 2).reshape(C, HL * D))]
        m = {"x8": x8, "xr8": xr8,
             "wps": np.ascontiguousarray(
                 Wp[:, g * HL * D:(g + 1) * HL * D].T).astype(bf),
             "tri": tri, "ones": ones, "shifti": shifti}
        for i, w in enumerate(ws):
            m[f"w8_{i}"], m[f"wr8_{i}"] = _split8(w * 64.0)
        in_maps.append(m)
    return in_maps


def assemble(results, bp):
    """Sum head-group partials per batch, add bias, transpose back."""
    out = np.empty((B, T, C), dtype=np.float32)
    for b in range(B):
        acc = (np.asarray(results[2 * b]["o"], dtype=np.float32)
               + np.asarray(results[2 * b + 1]["o"], dtype=np.float32))
        out[b] = acc.T + bp[None, :]
    return out


def kernel(x, Wq, Wk, Wv, Wp, bp):
    from concourse import bass_utils
    x = np.asarray(x, dtype=np.float32)
    nc = build_nc()
    in_maps = make_in_maps(np.asarray(x), np.asarray(Wq), np.asarray(Wk),
                           np.asarray(Wv), np.asarray(Wp))
    res = bass_utils.run_bass_kernel_spmd(nc, in_maps, core_ids=list(range(N_CORES)))
    return assemble(res.results, np.asarray(bp))


# revision 66
# speedup vs baseline: 1.0756x; 1.0756x over previous
"""Multi-head causal attention (B=4, T=2048, C=1024, H=16, D=64) on 8 trn2 cores.

Sharding: core c owns batch b = c//2 and heads g*8..g*8+7 where g = c%2
(batch-parallel x head-tensor-parallel). Each core computes its 8 heads'
QKV projections, causal attention, and a partial output projection
(columns of Wp belonging to its heads). Host sums the two head-group
partials per batch and adds the bias.

Dtypes (PSUM accumulation always f32):
 - QKV projections: fp8e4m3 value+residual splits of x and 64*W, prepared
   on the host. Three DoubleRow chains (x8*W8 + x8*Wr + xr8*W8) give
   near-bf16 accuracy at 0.5 cycles/row - 25% fewer PE cycles than bf16
   and 2x fewer than f32.
 - scores: fp8e4m3 Q/K with DoubleRow. Q/K are evacuated from the
   projection PSUM straight into tiles laid out [32 partitions, d-half, T]
   per head: the host orders Wq/Wk columns so head a's d-low rows land on
   partitions 32a..32a+31, making the evacuation partition-preserving.
   Matmul APs may only start at partitions 0/32/64, so the a=3 heads
   (partitions 96..127) get a small DMA shift into base-0 copies.
 - everything else (exp strips, V, AV, Y, Wp, o): bf16.
Measured end-to-end error vs the f32 reference ~8.6e-3 (tolerance 2e-2).

Scheduling: one global PE instruction stream, ordered so the ACT engine
(exp - the #2 engine at ~165us busy vs PE ~186us) starts by ~12us and is
never starved mid-stream. Projection and output-projection work is chopped
into ~1-2us units and woven between attention strip-passes as PE filler.
AV matmuls trail scores/exp by LAG strips (software pipeline); per-chunk
softmax normalization (1/rowsum broadcast via a rank-1 PE outer product)
runs as soon as a chunk's last strip lands. Normalized head outputs stay
in SBUF: even heads' rows land on partitions 0..63 directly; odd heads are
moved to partitions 64..127 with a matmul against a shifted identity
(DVE cannot cross partitions). The output projection reads Y from SBUF,
is woven into the second half-passes per chunk, and DMAs out o^T slices
as they complete.

Matmul shapes (out = lhsT.T @ rhs, contraction on partitions):
  QT/KT:  lhsT = W8[ckpair, 2, m-tile] rhs = x8[ckpair, 2, t-chunk] (DR)
  V:      lhsT = x8[ckpair, 2, s-tile] rhs = Wv8[ckpair, 2, :]      (DR)
  scoresT: lhsT = K2[32, 2, s-tile]    rhs = Q2[32, 2, t-seg]       (DR)
  AV^T:   lhsT = [V_h | 1][s-tile,65]  rhs = expT strip [s-tile, t]
  out^T:  lhsT = WpS[j-tile, c-tile]   rhs = Y[j-tile, t-chunk]
"""

import numpy as np
import ml_dtypes
from contextlib import ExitStack

B, T, C, H, D = 4, 2048, 1024, 16, 64
HL = H // 2          # 8 heads per core
N_CORES = 8
P = 128
NK = C // P          # 8 contraction tiles for projections
NM = HL * D // P     # 4 m-tiles of Q/K head-dims
NS = T // P          # 16 s-tiles (key strips)
CH = 512             # t-chunk width
NCH = T // CH        # 4 t-chunks
LAG = 14             # AV trails scores by this many strips

_nc_cache = None


def build_nc():
    global _nc_cache
    if _nc_cache is not None:
        return _nc_cache
    import concourse.bass as bass  # noqa: F401
    import concourse.tile as tile
    from concourse import bacc, mybir

    f32 = mybir.dt.float32
    bf16 = mybir.dt.bfloat16
    fp8 = mybir.dt.float8e4
    Exp = mybir.ActivationFunctionType.Exp
    DR = mybir.MatmulPerfMode.DoubleRow

    def mm(out, lhsT, rhs, **kw):
        nc.tensor.matmul(out, lhsT=lhsT, rhs=rhs, **kw)

    nc = bacc.Bacc("TRN2", target_bir_lowering=False, debug=False,
                   enable_asserts=True, num_devices=N_CORES)
    # x and the QKV weights ship as fp8 value+residual pairs, partition-
    # major: x as [P, NK/2, 2(k-pair), T], W as [P, NK/2, 2, 512], weights
    # prescaled by 64 (psum then holds 64*Q etc.)
    NKP = NK // 2
    x8 = nc.dram_tensor("x8", (P, NKP, 2, T), fp8, kind="ExternalInput").ap()
    xr8 = nc.dram_tensor("xr8", (P, NKP, 2, T), fp8, kind="ExternalInput").ap()
    w8 = [nc.dram_tensor(f"w8_{i}", (P, NKP, 2, HL * D), fp8,
                         kind="ExternalInput").ap() for i in range(3)]
    wr8 = [nc.dram_tensor(f"wr8_{i}", (P, NKP, 2, HL * D), fp8,
                          kind="ExternalInput").ap() for i in range(3)]
    wps = nc.dram_tensor("wps", (HL * D, C), bf16, kind="ExternalInput").ap()
    tri = nc.dram_tensor("tri", (P, 2 * P), bf16, kind="ExternalInput").ap()
    ones = nc.dram_tensor("ones", (P, 2 * P), bf16, kind="ExternalInput").ap()
    # shifti[k, 64+k] = 1: a matmul against it moves rows down 64 partitions
    shifti = nc.dram_tensor("shifti", (D, P), bf16, kind="ExternalInput").ap()
    o = nc.dram_tensor("o", (C, T), bf16, kind="ExternalOutput").ap()

    with tile.TileContext(nc) as tc:
        with ExitStack() as ctx:
            ctx.enter_context(nc.allow_low_precision(
                reason="bf16/fp8 operands with f32 PSUM accumulation"))
            # PSUM banks (8 total): scores 2x[128,1024]=4, small-mm
            # (proj/outproj/rank1/shift) 2x[128,512]=2, av 2x[65,512]=2.
            # Separate tags so filler units never block the scores->exp
            # pipeline's psum rotation.
            sc_ps = ctx.enter_context(tc.tile_pool(name="sc_ps", bufs=2, space="PSUM"))
            sm_ps = ctx.enter_context(tc.tile_pool(name="sm_ps", bufs=2, space="PSUM"))
            av_ps = ctx.enter_context(tc.tile_pool(name="av_ps", bufs=2, space="PSUM"))

            const_pool = ctx.enter_context(tc.tile_pool(name="const", bufs=1))
            tri_sb = const_pool.tile([P, 2 * P], bf16, name="tri_sb", tag="tri_sb")
            ones_sb = const_pool.tile([P, D], bf16, name="ones_sb", tag="ones_sb")
            shifti_sb = const_pool.tile([D, P], bf16, name="shifti_sb",
                                        tag="shifti_sb")

            main = ctx.enter_context(tc.tile_pool(name="main", bufs=1))
            # Q2/K2: [partition 32a+dl, g, u(d-half), t] fp8 - head h=4g+a's
            # (d = 32u+dl) value lives at partition 32a+dl, free (g, u, t).
            # Matmul APs may only start at partitions 0/32/64, so the a=3
            # heads (partitions 96..127) are DMA-shifted into base-0 copies.
            Q2 = main.tile([P, 2, 2, T], fp8, name="q2", tag="q2")
            K2 = main.tile([P, 2, 2, T], fp8, name="k2", tag="k2")
            Q2x = main.tile([32, 2, 2, T], fp8, name="q2x", tag="q2x")
            K2x = main.tile([32, 2, 2, T], fp8, name="k2x", tag="k2x")
            # V: [s-within-tile, s-tile, head, d+1]; col 64 = ones (rowsum)
            Vsb = main.tile([P, NS, HL, D + 1], bf16, name="vsb", tag="vsb")
            Wp_sb = [main.tile([P, C], bf16, name=f"wp{j}", tag=f"wp{j}")
                     for j in range(NM)]
            # normalized head outputs YT, SBUF-resident: tile m holds heads
            # 2m (partitions 0..63) and 2m+1 (64..127)
            Ysb = [main.tile([P, T], bf16, name=f"y{m}", tag=f"y{m}")
                   for m in range(NM)]

            wpool = ctx.enter_context(tc.tile_pool(name="wpool", bufs=1))
            # one consolidated DMA per weight tensor: [p, k-pair, 2, m]
            W_sb = [wpool.tile([P, NKP, 2, HL * D], fp8, name=f"w8{pj}",
                               tag=f"w8{pj}") for pj in range(3)]
            Wr_sb = [wpool.tile([P, NKP, 2, HL * D], fp8, name=f"wr{pj}",
                                tag=f"wr{pj}") for pj in range(3)]

            def dma_w(pj, eng):
                eng.dma_start(out=W_sb[pj], in_=w8[pj])
                eng.dma_start(out=Wr_sb[pj], in_=wr8[pj])

            xpool = ctx.enter_context(tc.tile_pool(name="xpool", bufs=2))
            xs_tiles = {}

            def dma_x(ch, eng=None, split=False):
                xs = xpool.tile([P, NKP, 2, CH], fp8, name="xs", tag="xs")
                xr = xpool.tile([P, NKP, 2, CH], fp8, name="xr", tag="xr")
                e = eng or nc.scalar
                e.dma_start(out=xs, in_=x8[:, :, :, ch * CH:(ch + 1) * CH])
                if not split:
                    e.dma_start(out=xr, in_=xr8[:, :, :, ch * CH:(ch + 1) * CH])
                    xs_tiles[ch] = (xs, xr)
                    return
                xs_tiles[ch] = (xs, xr)

                def later():
                    e.dma_start(out=xr, in_=xr8[:, :, :, ch * CH:(ch + 1) * CH])
                return later

            strip_pool = ctx.enter_context(tc.tile_pool(name="strip_pool", bufs=18))
            small = ctx.enter_context(tc.tile_pool(name="small", bufs=4))
            tmp_pool = ctx.enter_context(tc.tile_pool(name="tmp_pool", bufs=4))
            obpool = ctx.enter_context(tc.tile_pool(name="obpool", bufs=6))

            # ---------------- work units ----------------
            def pu_qk(proj, m, ch):
                """Q or K projection unit -> fp8 split tiles. m = 2g+u.
                Split-fp8: psum = x8*W8 + x8*Wr + xr8*W8 ~= 64*Q exactly."""
                def run():
                    xs, xr = xs_tiles[ch]
                    dst = Q2 if proj == 0 else K2
                    g, u = m // 2, m % 2
                    ms = slice(m * P, (m + 1) * P)
                    ps = sm_ps.tile([P, CH], f32, name="qk_ps", tag="sm")
                    first = True
                    for wt, xt in ((W_sb[proj], xs), (Wr_sb[proj], xs),
                                   (W_sb[proj], xr)):
                        for kp in range(NKP):
                            mm(ps, wt[:, kp, :, ms], xt[:, kp, :, :],
                               perf_mode=DR, start=first,
                               stop=(not first and wt is W_sb[proj]
                                     and xt is xr and kp == NKP - 1))
                            first = False
                    dslice = dst[:, g, u, ch * CH:(ch + 1) * CH]
                    if ch == 2:
                        nc.vector.tensor_copy(dslice, ps)
                    else:
                        nc.scalar.copy(dslice, ps)
                return run

            def pu_remap(proj, g, ch):
                """Shift head a=3's (g, ch) slice down to the base-0 copy."""
                def run():
                    src, dst = (Q2, Q2x) if proj == 0 else (K2, K2x)
                    nc.sync.dma_start(
                        out=dst[:, g, :, ch * CH:(ch + 1) * CH],
                        in_=src[96:128, g, :, ch * CH:(ch + 1) * CH])
                return run

            def pu_v(s):
                """V projection unit for s-tile s (psum = 64*V; evac /64)."""
                def run():
                    ch, sl = s // (CH // P), s % (CH // P)
                    xs, xr = xs_tiles[ch]
                    ss = slice(sl * P, (sl + 1) * P)
                    ps = sm_ps.tile([P, HL * D], f32, name="v_ps", tag="sm")
                    first = True
                    chains = ((xs, W_sb[2]), (xs, Wr_sb[2]), (xr, W_sb[2]))
                    for ci, (xt, wt) in enumerate(chains):
                        for kp in range(NKP):
                            mm(ps, xt[:, kp, :, ss], wt[:, kp, :, :],
                               perf_mode=DR, start=first,
                               stop=(ci == 2 and kp == NKP - 1))
                            first = False
                    # evac on ACT where it idles (pre-pass region);
                    # on DVE in G1 where ACT is the exp bottleneck. The
                    # 1/64 de-scale fuses into either copy.
                    if s < 8:
                        nc.scalar.activation(
                            Vsb[:, s, :, 0:D],
                            ps.rearrange("p (h d) -> p h d", h=HL),
                            mybir.ActivationFunctionType.Copy, scale=1.0 / 64)
                    else:
                        nc.vector.tensor_scalar_mul(
                            Vsb[:, s, :, 0:D],
                            ps.rearrange("p (h d) -> p h d", h=HL), 1.0 / 64)
                return run

            def op_unit(ct, ch):
                """Output-projection unit: o^T[ct-tile, ch-chunk]."""
                def run():
                    ps = sm_ps.tile([P, CH], f32, name="p_ps", tag="sm")
                    for j in range(NM):
                        mm(ps, Wp_sb[j][:, ct * P:(ct + 1) * P],
                           Ysb[j][:, ch * CH:(ch + 1) * CH],
                           start=(j == 0), stop=(j == NM - 1))
                    ob = obpool.tile([P, CH], bf16, name="ob", tag="ob")
                    if ch >= 2:
                        # tail chunks run after the last exp - ACT is free
                        nc.scalar.copy(ob, ps)
                    else:
                        nc.vector.tensor_copy(ob, ps)
                    eng = (nc.sync, nc.gpsimd)[(ct + ch) % 2]
                    eng.dma_start(
                        out=o[ct * P:(ct + 1) * P, ch * CH:(ch + 1) * CH], in_=ob)
                return run

            # ---------------- attention passes ----------------
            def make_pass(h, half):
                g, a = h // 4, h % 4
                tlo = half * 1024
                ns = 8 if half == 0 else NS
                st = {"strips": [None] * ns, "avs": None}

                if a < 3:
                    qt, kt, pbase = Q2, K2, 32 * a
                else:
                    qt, kt, pbase = Q2x, K2x, 0

                def do_scores(i):
                    t0 = P * i
                    s0 = max(t0, tlo)
                    strip = strip_pool.tile([P, 1024], bf16,
                                            name="strip", tag="strip")
                    st["strips"][i] = strip
                    ps = sc_ps.tile([P, 1024], f32, name="sc_ps", tag="sc")
                    b0 = s0
                    while b0 < tlo + 1024:
                        b1 = min((b0 // CH + 1) * CH, tlo + 1024)
                        mm(ps[:, b0 - tlo:b1 - tlo],
                           kt[pbase:pbase + 32, g, :, t0:t0 + P],
                           qt[pbase:pbase + 32, g, :, b0:b1],
                           perf_mode=DR, start=True, stop=True)
                        b0 = b1
                    nc.scalar.activation(
                        strip[:, s0 - tlo:1024], ps[:, s0 - tlo:1024],
                        Exp, scale=float(1.0 / np.sqrt(D) / 4096))

                def do_av(i):
                    if st["avs"] is None:
                        st["avs"] = {j: av_ps.tile([D + 1, CH], f32,
                                                   name=f"av{j}", tag="av")
                                     for j in (2 * half, 2 * half + 1)}
                    avs = st["avs"]
                    t0 = P * i
                    strip = st["strips"][i]
                    if t0 >= tlo:              # diagonal block: causal mask
                        nc.vector.tensor_mul(
                            strip[:, t0 - tlo:t0 + P - tlo],
                            strip[:, t0 - tlo:t0 + P - tlo],
                            tri_sb[:, P:2 * P])
                    for j in (2 * half, 2 * half + 1):
                        if CH * (j + 1) <= t0:
                            continue
                        ts0 = max(CH * j, t0)
                        mm(avs[j][:, ts0 - CH * j:CH],
                           Vsb[:, i, h, :],
                           strip[:, ts0 - tlo:CH * (j + 1) - tlo],
                           start=(i == 0), stop=(i == 4 * j + 3),
                           skip_group_check=True)
                    if i % 4 == 3 and i // 4 in avs:
                        j = i // 4
                        # a single copy frees the scarce AV psum slot right
                        # away; the rest of the normalize runs off-psum, and
                        # the multiply can then read the rank-1 broadcast
                        # PSUM directly (only one PSUM operand per DVE op)
                        avc = small.tile([D + 1, CH], bf16, name="avc",
                                         tag="avc")
                        nc.vector.tensor_copy(avc, avs[j])
                        rec = small.tile([D + 1, CH], bf16, name="rec", tag="rec")
                        nc.vector.reciprocal(rec[D:D + 1, :], avc[D:D + 1, :])
                        rps = sm_ps.tile([P, CH], f32, name="rps", tag="sm")
                        mm(rps[0:D, :], ones_sb[D:D + 1, 0:D], rec[D:D + 1, :],
                           start=True, stop=True)
                        if h % 2 == 0:
                            # even head: rows land at partitions 0..63 directly
                            nc.vector.tensor_mul(
                                Ysb[h // 2][0:D, CH * j:CH * (j + 1)],
                                avc[0:D, :], rps[0:D, :])
                        else:
                            # odd head: normalize into tmp, then a matmul
                            # against shifti moves the rows to partitions
                            # 64..127 (DVE cannot cross partitions)
                            tmpc = tmp_pool.tile([D, CH], bf16,
                                                 name="tmpc", tag="tmpc")
                            nc.vector.tensor_mul(tmpc, avc[0:D, :],
                                                 rps[0:D, :])
                            shp = sm_ps.tile([P, CH], f32, name="shp", tag="sm")
                            mm(shp, shifti_sb, tmpc, start=True, stop=True)
                            nc.vector.tensor_copy(
                                Ysb[h // 2][D:P, CH * j:CH * (j + 1)],
                                shp[D:P, :])

                return ([lambda i=i: do_scores(i) for i in range(ns)],
                        [lambda i=i: do_av(i) for i in range(ns)])

            # ---------------- global schedule ----------------
            # entries: ("s", scores_fn) participates in the LAG pipeline;
            #          ("f", filler_fn) is PE filler / DMA issue.
            entries = []
            avq = []

            def weave(pass_list, fillers, per, start_after=0):
                """Interleave `fillers` into the passes, `per` fillers after
                each scores entry (fractional via accumulator). The first
                filler is held back until `start_after` scores entries."""
                fi = -per * start_after
                fidx = 0
                for sc, ac in pass_list:
                    for s_fn, a_fn in zip(sc, ac):
                        entries.append(("s", s_fn))
                        avq.append(a_fn)
                        fi += per
                        while fidx + 1 <= fi and fidx < len(fillers):
                            entries.append(("f", fillers[fidx]))
                            fidx += 1
                while fidx < len(fillers):
                    entries.append(("f", fillers[fidx]))
                    fidx += 1

            # setup DMAs in priority order, spread across engine DGE queues:
            # the first projection units need x ch0 + wq/wk only.
            xr0_later = dma_x(0, nc.scalar, split=True)
            dma_w(0, nc.sync)
            xr0_later()
            dma_w(1, nc.gpsimd)
            dma_x(1, nc.sync)
            dma_w(2, nc.scalar)           # wv: needed from V s0 (~unit 9)
            nc.sync.dma_start(
                out=Vsb[:, :, :, D],
                in_=ones[:, 0:NS * HL].rearrange("p (s h) -> p s h", s=NS))
            nc.scalar.dma_start(out=tri_sb, in_=tri)
            nc.scalar.dma_start(out=ones_sb, in_=ones[:, 0:D])
            nc.gpsimd.dma_start(out=shifti_sb, in_=shifti)
            for j in range(NM):
                nc.gpsimd.dma_start(out=Wp_sb[j], in_=wps[j * P:(j + 1) * P, :])

            # minimal prologue: Q/K m0/m1 (heads 0-3) for chunks 0,1 plus the
            # first two V tiles; everything else weaves into the passes.
            pro = []
            for ch in (0, 1):
                for m in (0, 1):
                    pro.append(pu_qk(0, m, ch))
                    pro.append(pu_qk(1, m, ch))
            for ch in (0, 1):
                pro.append(pu_remap(0, 0, ch))
                pro.append(pu_remap(1, 0, ch))
            pro += [pu_v(0), pu_v(1)]
            for f in pro:
                entries.append(("f", f))

            passes0 = [make_pass(h, 0) for h in range(HL)]
            g1_order = [1, 3, 5, 7, 0, 2, 4, 6]
            passes1 = [make_pass(h, 1) for h in g1_order]

            # G0a: first half-0 pass, densely packed with the remaining V
            # tiles its AVs need (V s-tile i is read at stream index i+LAG)
            weave(passes0[:1], [pu_v(s) for s in range(2, 8)], per=6 / 8.0)

            # G0b: remaining half-0 passes; fillers = Q/K m2/m3 (heads 4-7,
            # needed by pass h4 at scores index 24 of this group), chunk-2
            # projections, x2/x3 DMAs
            g0_fill = []
            for m in (2, 3):
                for ch in (0, 1):
                    g0_fill.append(pu_qk(0, m, ch))
                    g0_fill.append(pu_qk(1, m, ch))
            for ch in (0, 1):
                g0_fill.append(pu_remap(0, 1, ch))
                g0_fill.append(pu_remap(1, 1, ch))
            g0_fill.append(lambda: dma_x(2))
            for m in range(NM):
                g0_fill.append(pu_qk(0, m, 2))
                g0_fill.append(pu_qk(1, m, 2))
                if m % 2 == 1:
                    g0_fill.append(pu_remap(0, m // 2, 2))
                    g0_fill.append(pu_remap(1, m // 2, 2))
            g0_fill.append(lambda: dma_x(3))
            # chunk-3 Q/K must complete before any half-1 scores (their
            # first segments read ch3 columns); weaving them as the final
            # G0b fillers preserves that while keeping ACT fed
            for m in range(NM):
                g0_fill.append(pu_qk(0, m, 3))
                g0_fill.append(pu_qk(1, m, 3))
                if m % 2 == 1:
                    g0_fill.append(pu_remap(0, m // 2, 3))
                    g0_fill.append(pu_remap(1, m // 2, 3))
            weave(passes0[1:], g0_fill, per=len(g0_fill) / 56.0)
            # G1a: first half-1 pass densely packed with V s8..15 (V
            # s-tile i is first read by AV(i) at stream index i+LAG, so a
            # 1-per-scores weave lands each tile well before its reader)
            weave(passes1[:1], [pu_v(s) for s in range(8, 16)], per=1.0)

            # G1b: remaining half-1 passes; fillers = chunk-0/1 output
            # projection (ready once all half-0 passes have fully drained
            # through the LAG pipeline - hold fillers back past that point)
            g1_fill = [op_unit(ct, 0) for ct in range(NK)]
            g1_fill += [op_unit(ct, 1) for ct in range(NK)]
            weave(passes1[1:], g1_fill, per=len(g1_fill) / 64.0,
                  start_after=2)

            # tail: chunk-2 output projection can start as soon as the last
            # pass's strip-11 AV (chunk-2 normalize, avq index 187) drains;
            # chunk-3 needs the full drain.
            inject = {64 + 7 * 16 + 11: [op_unit(ct, 2) for ct in range(NK)]}
            tail = [op_unit(ct, 3) for ct in range(NK)]

            # ---------------- execute the stream ----------------
            si = 0
            for kind, fn in entries:
                if kind == "f":
                    fn()
                    continue
                fn()
                if si >= LAG:
                    avq[si - LAG]()
                si += 1
            for k in range(max(si - LAG, 0), si):
                avq[k]()
                for fn in inject.get(k, ()):
                    fn()
            for fn in tail:
                fn()

    nc.compile()
    _nc_cache = nc
    return nc


def _split_qk(Warr):
    """[8, C, 64] -> [C, 512] with column m' = 128*(2g+u) + 32a + dl for
    head h=4g+a, d=32u+dl (matches the Q2/K2 on-device layout)."""
    arr = Warr.reshape(2, 4, C, 2, 32)        # [g, a, c, u, dl]
    arr = arr.transpose(2, 0, 3, 1, 4)        # [c, g, u, a, dl]
    return np.ascontiguousarray(arr.reshape(C, HL * D))


F8 = ml_dtypes.float8_e4m3


def _q8(v):
    return np.asarray(v, F8)


def _pair_layout(arr):
    """[C, m] -> [P, NK/2, 2, m] (contraction split into DoubleRow pairs)."""
    m = arr.shape[1]
    return np.ascontiguousarray(
        arr.reshape(NK // 2, 2, P, m).transpose(2, 0, 1, 3))


def _split8(arr):
    """fp8 value+residual pair of [C, m] array, in pair layout."""
    a8 = _q8(arr)
    r8 = _q8(arr - a8.astype(np.float32))
    return _pair_layout(a8), _pair_layout(r8)


def make_in_maps(x, Wq, Wk, Wv, Wp):
    """Shard FULL inputs into per-core input maps."""
    bf = ml_dtypes.bfloat16
    tri = np.concatenate(
        [np.zeros((P, P), dtype=np.float32),
         np.triu(np.ones((P, P), dtype=np.float32))], axis=1).astype(bf)
    ones = np.ones((P, 2 * P), dtype=np.float32).astype(bf)
    shifti = np.zeros((D, P), dtype=np.float32)
    shifti[np.arange(D), D + np.arange(D)] = 1.0
    shifti = shifti.astype(bf)
    in_maps = []
    for c in range(N_CORES):
        b, g = c // 2, c % 2
        hs = slice(g * HL, (g + 1) * HL)
        x8, xr8 = _split8(np.ascontiguousarray(x[b].T))
        ws = [_split_qk(Wq[hs]), _split_qk(Wk[hs]),
              np.ascontiguousarray(
                  Wv[hs].transpose(1, 0, 2).reshape(C, HL * D))]
        m = {"x8": x8, "xr8": xr8,
             "wps": np.ascontiguousarray(
                 Wp[:, g * HL * D:(g + 1) * HL * D].T).astype(bf),
             "tri": tri, "ones": ones, "shifti": shifti}
        for i, w in enumerate(ws):
            m[f"w8_{i}"], m[f"wr8_{i}"] = _split8(w * 64.0)
        in_maps.append(m)
    return in_maps


def assemble(results, bp):
    """Sum head-group partials per batch, add bias, transpose back."""
    out = np.empty((B, T, C), dtype=np.float32)
    for b in range(B):
        acc = (np.asarray(results[2 * b]["o"], dtype=np.float32)
               + np.asarray(results[2 * b + 1]["o"], dtype=np.float32))
        out[b] = acc.T + bp[None, :]
    return out


def kernel(x, Wq, Wk, Wv, Wp, bp):
    from concourse import bass_utils
    x = np.asarray(x, dtype=np.float32)
    nc = build_nc()
    in_maps = make_in_maps(np.asarray(x), np.asarray(Wq), np.asarray(Wk),
                           np.asarray(Wv), np.asarray(Wp))
    res = bass_utils.run_bass_kernel_spmd(nc, in_maps, core_ids=list(range(N_CORES)))
    return assemble(res.results, np.asarray(bp))


# revision 76
# speedup vs baseline: 1.0913x; 1.0147x over previous
"""Multi-head causal attention (B=4, T=2048, C=1024, H=16, D=64) on 8 trn2 cores.

Sharding: core c owns batch b = c//2 and heads g*8..g*8+7 where g = c%2
(batch-parallel x head-tensor-parallel). Each core computes its 8 heads'
QKV projections, causal attention, and a partial output projection
(columns of Wp belonging to its heads). Host sums the two head-group
partials per batch and adds the bias.

Dtypes (PSUM accumulation always f32):
 - QKV projections: fp8e4m3 value+residual splits of x and 64*W, prepared
   on the host. Three DoubleRow chains (x8*W8 + x8*Wr + xr8*W8) give
   near-bf16 accuracy at 0.5 cycles/row - 25% fewer PE cycles than bf16
   and 2x fewer than f32.
 - scores: fp8e4m3 Q/K with DoubleRow. Q/K are evacuated from the
   projection PSUM straight into tiles laid out [32 partitions, d-half, T]
   per head: the host orders Wq/Wk columns so head a's d-low rows land on
   partitions 32a..32a+31, making the evacuation partition-preserving.
   Matmul APs may only start at partitions 0/32/64, so the a=3 heads
   (partitions 96..127) get a small DMA shift into base-0 copies.
 - everything else (exp strips, V, AV, Y, Wp, o): bf16.
Measured end-to-end error vs the f32 reference ~8.6e-3 (tolerance 2e-2).

Scheduling: one global PE instruction stream, ordered so the ACT engine
(exp - the #2 engine at ~165us busy vs PE ~186us) starts by ~12us and is
never starved mid-stream. Projection and output-projection work is chopped
into ~1-2us units and woven between attention strip-passes as PE filler.
AV matmuls trail scores/exp by LAG strips (software pipeline); per-chunk
softmax normalization (1/rowsum broadcast via a rank-1 PE outer product)
runs as soon as a chunk's last strip lands. Normalized head outputs stay
in SBUF: even heads' rows land on partitions 0..63 directly; odd heads are
moved to partitions 64..127 with a matmul against a shifted identity
(DVE cannot cross partitions). The output projection reads Y from SBUF,
is woven into the second half-passes per chunk, and DMAs out o^T slices
as they complete.

Matmul shapes (out = lhsT.T @ rhs, contraction on partitions):
  QT/KT:  lhsT = W8[ckpair, 2, m-tile] rhs = x8[ckpair, 2, t-chunk] (DR)
  V:      lhsT = x8[ckpair, 2, s-tile] rhs = Wv8[ckpair, 2, :]      (DR)
  scoresT: lhsT = K2[32, 2, s-tile]    rhs = Q2[32, 2, t-seg]       (DR)
  AV^T:   lhsT = [V_h | 1][s-tile,65]  rhs = expT strip [s-tile, t]
  out^T:  lhsT = WpS[j-tile, c-tile]   rhs = Y[j-tile, t-chunk]
"""

import numpy as np
import ml_dtypes
from contextlib import ExitStack

B, T, C, H, D = 4, 2048, 1024, 16, 64
HL = H // 2          # 8 heads per core
N_CORES = 8
P = 128
NK = C // P          # 8 contraction tiles for projections
NM = HL * D // P     # 4 m-tiles of Q/K head-dims
NS = T // P          # 16 s-tiles (key strips)
CH = 512             # t-chunk width
NCH = T // CH        # 4 t-chunks
LAG = 30             # AV trails scores by this many strips

_nc_cache = None


def build_nc():
    global _nc_cache
    if _nc_cache is not None:
        return _nc_cache
    import concourse.bass as bass  # noqa: F401
    import concourse.tile as tile
    from concourse import bacc, mybir

    f32 = mybir.dt.float32
    bf16 = mybir.dt.bfloat16
    fp8 = mybir.dt.float8e4
    Exp = mybir.ActivationFunctionType.Exp
    DR = mybir.MatmulPerfMode.DoubleRow

    def mm(out, lhsT, rhs, **kw):
        nc.tensor.matmul(out, lhsT=lhsT, rhs=rhs, **kw)

    nc = bacc.Bacc("TRN2", target_bir_lowering=False, debug=False,
                   enable_asserts=True, num_devices=N_CORES)
    # x and the QKV weights ship as fp8 value+residual pairs, partition-
    # major: x as [P, NK/2, 2(k-pair), T], W as [P, NK/2, 2, 512], weights
    # prescaled by 64 (psum then holds 64*Q etc.)
    NKP = NK // 2
    x8 = nc.dram_tensor("x8", (P, NKP, 2, T), fp8, kind="ExternalInput").ap()
    xr8 = nc.dram_tensor("xr8", (P, NKP, 2, T), fp8, kind="ExternalInput").ap()
    w8 = [nc.dram_tensor(f"w8_{i}", (P, NKP, 2, HL * D), fp8,
                         kind="ExternalInput").ap() for i in range(3)]
    wr8 = [nc.dram_tensor(f"wr8_{i}", (P, NKP, 2, HL * D), fp8,
                          kind="ExternalInput").ap() for i in range(3)]
    wps = nc.dram_tensor("wps", (HL * D, C), bf16, kind="ExternalInput").ap()
    tri = nc.dram_tensor("tri", (P, 2 * P), bf16, kind="ExternalInput").ap()
    ones = nc.dram_tensor("ones", (P, 2 * P), bf16, kind="ExternalInput").ap()
    # shifti[k, 64+k] = 1: a matmul against it moves rows down 64 partitions
    shifti = nc.dram_tensor("shifti", (D, P), bf16, kind="ExternalInput").ap()
    o = nc.dram_tensor("o", (C, T), bf16, kind="ExternalOutput").ap()

    with tile.TileContext(nc) as tc:
        with ExitStack() as ctx:
            ctx.enter_context(nc.allow_low_precision(
                reason="bf16/fp8 operands with f32 PSUM accumulation"))
            # PSUM banks (8 total): scores 2x[128,1024]=4, small-mm
            # (proj/outproj/rank1/shift) 2x[128,512]=2, av 2x[65,512]=2.
            # Separate tags so filler units never block the scores->exp
            # pipeline's psum rotation.
            sc_ps = ctx.enter_context(tc.tile_pool(name="sc_ps", bufs=2, space="PSUM"))
            sm_ps = ctx.enter_context(tc.tile_pool(name="sm_ps", bufs=2, space="PSUM"))
            av_ps = ctx.enter_context(tc.tile_pool(name="av_ps", bufs=2, space="PSUM"))

            const_pool = ctx.enter_context(tc.tile_pool(name="const", bufs=1))
            tri_sb = const_pool.tile([P, 2 * P], bf16, name="tri_sb", tag="tri_sb")
            ones_sb = const_pool.tile([P, D], bf16, name="ones_sb", tag="ones_sb")
            shifti_sb = const_pool.tile([D, P], bf16, name="shifti_sb",
                                        tag="shifti_sb")

            main = ctx.enter_context(tc.tile_pool(name="main", bufs=1))
            # Q2/K2: [partition 32a+dl, g, u(d-half), t] fp8 - head h=4g+a's
            # (d = 32u+dl) value lives at partition 32a+dl, free (g, u, t).
            # Matmul APs may only start at partitions 0/32/64, so the a=3
            # heads (partitions 96..127) are DMA-shifted into base-0 copies.
            Q2 = main.tile([P, 2, 2, T], fp8, name="q2", tag="q2")
            K2 = main.tile([P, 2, 2, T], fp8, name="k2", tag="k2")
            Q2x = main.tile([32, 2, 2, T], fp8, name="q2x", tag="q2x")
            K2x = main.tile([32, 2, 2, T], fp8, name="k2x", tag="k2x")
            # V: [s-within-tile, s-tile, head, d+1]; col 64 = ones (rowsum)
            Vsb = main.tile([P, NS, HL, D + 1], bf16, name="vsb", tag="vsb")
            Wp_sb = [main.tile([P, C], bf16, name=f"wp{j}", tag=f"wp{j}")
                     for j in range(NM)]
            # normalized head outputs YT, SBUF-resident: tile m holds heads
            # 2m (partitions 0..63) and 2m+1 (64..127)
            Ysb = [main.tile([P, T], bf16, name=f"y{m}", tag=f"y{m}")
                   for m in range(NM)]

            wpool = ctx.enter_context(tc.tile_pool(name="wpool", bufs=1))
            # one consolidated DMA per weight tensor: [p, k-pair, 2, m]
            W_sb = [wpool.tile([P, NKP, 2, HL * D], fp8, name=f"w8{pj}",
                               tag=f"w8{pj}") for pj in range(3)]
            Wr_sb = [wpool.tile([P, NKP, 2, HL * D], fp8, name=f"wr{pj}",
                                tag=f"wr{pj}") for pj in range(3)]

            def dma_w(pj, eng):
                eng.dma_start(out=W_sb[pj], in_=w8[pj])
                eng.dma_start(out=Wr_sb[pj], in_=wr8[pj])

            xpool = ctx.enter_context(tc.tile_pool(name="xpool", bufs=2))
            xs_tiles = {}

            def dma_x(ch, eng=None, split=False):
                xs = xpool.tile([P, NKP, 2, CH], fp8, name="xs", tag="xs")
                xr = xpool.tile([P, NKP, 2, CH], fp8, name="xr", tag="xr")
                e = eng or nc.scalar
                e.dma_start(out=xs, in_=x8[:, :, :, ch * CH:(ch + 1) * CH])
                if not split:
                    e.dma_start(out=xr, in_=xr8[:, :, :, ch * CH:(ch + 1) * CH])
                    xs_tiles[ch] = (xs, xr)
                    return
                xs_tiles[ch] = (xs, xr)

                def later():
                    e.dma_start(out=xr, in_=xr8[:, :, :, ch * CH:(ch + 1) * CH])
                return later

            strip_pool = ctx.enter_context(tc.tile_pool(name="strip_pool", bufs=34))
            small = ctx.enter_context(tc.tile_pool(name="small", bufs=4))
            tmp_pool = ctx.enter_context(tc.tile_pool(name="tmp_pool", bufs=4))
            obpool = ctx.enter_context(tc.tile_pool(name="obpool", bufs=6))

            # ---------------- work units ----------------
            def pu_qk(proj, m, ch):
                """Q or K projection unit -> fp8 split tiles. m = 2g+u.
                Split-fp8: psum = x8*W8 + x8*Wr + xr8*W8 ~= 64*Q exactly."""
                def run():
                    xs, xr = xs_tiles[ch]
                    dst = Q2 if proj == 0 else K2
                    g, u = m // 2, m % 2
                    ms = slice(m * P, (m + 1) * P)
                    ps = sm_ps.tile([P, CH], f32, name="qk_ps", tag="sm")
                    first = True
                    for wt, xt in ((W_sb[proj], xs), (Wr_sb[proj], xs),
                                   (W_sb[proj], xr)):
                        for kp in range(NKP):
                            mm(ps, wt[:, kp, :, ms], xt[:, kp, :, :],
                               perf_mode=DR, start=first,
                               stop=(not first and wt is W_sb[proj]
                                     and xt is xr and kp == NKP - 1))
                            first = False
                    dslice = dst[:, g, u, ch * CH:(ch + 1) * CH]
                    if ch == 2:
                        nc.vector.tensor_copy(dslice, ps)
                    else:
                        nc.scalar.copy(dslice, ps)
                return run

            def pu_remap(proj, g, ch):
                """Shift head a=3's (g, ch) slice down to the base-0 copy."""
                def run():
                    src, dst = (Q2, Q2x) if proj == 0 else (K2, K2x)
                    nc.sync.dma_start(
                        out=dst[:, g, :, ch * CH:(ch + 1) * CH],
                        in_=src[96:128, g, :, ch * CH:(ch + 1) * CH])
                return run

            def pu_v(s):
                """V projection unit for s-tile s (psum = 64*V; evac /64)."""
                def run():
                    ch, sl = s // (CH // P), s % (CH // P)
                    xs, xr = xs_tiles[ch]
                    ss = slice(sl * P, (sl + 1) * P)
                    ps = sm_ps.tile([P, HL * D], f32, name="v_ps", tag="sm")
                    first = True
                    chains = ((xs, W_sb[2]), (xs, Wr_sb[2]), (xr, W_sb[2]))
                    for ci, (xt, wt) in enumerate(chains):
                        for kp in range(NKP):
                            mm(ps, xt[:, kp, :, ss], wt[:, kp, :, :],
                               perf_mode=DR, start=first,
                               stop=(ci == 2 and kp == NKP - 1))
                            first = False
                    # evac on ACT where it idles (pre-pass region);
                    # on DVE in G1 where ACT is the exp bottleneck. The
                    # 1/64 de-scale fuses into either copy.
                    if s < 8:
                        nc.scalar.activation(
                            Vsb[:, s, :, 0:D],
                            ps.rearrange("p (h d) -> p h d", h=HL),
                            mybir.ActivationFunctionType.Copy, scale=1.0 / 64)
                    else:
                        nc.vector.tensor_scalar_mul(
                            Vsb[:, s, :, 0:D],
                            ps.rearrange("p (h d) -> p h d", h=HL), 1.0 / 64)
                return run

            def op_unit(ct, ch):
                """Output-projection unit: o^T[ct-tile, ch-chunk]."""
                def run():
                    ps = sm_ps.tile([P, CH], f32, name="p_ps", tag="sm")
                    for j in range(NM):
                        mm(ps, Wp_sb[j][:, ct * P:(ct + 1) * P],
                           Ysb[j][:, ch * CH:(ch + 1) * CH],
                           start=(j == 0), stop=(j == NM - 1))
                    ob = obpool.tile([P, CH], bf16, name="ob", tag="ob")
                    if ch >= 2:
                        # tail chunks run after the last exp - ACT is free
                        nc.scalar.copy(ob, ps)
                    else:
                        nc.vector.tensor_copy(ob, ps)
                    eng = (nc.sync, nc.gpsimd)[(ct + ch) % 2]
                    eng.dma_start(
                        out=o[ct * P:(ct + 1) * P, ch * CH:(ch + 1) * CH], in_=ob)
                return run

            # ---------------- attention passes ----------------
            def make_pass(h, half):
                g, a = h // 4, h % 4
                tlo = half * 1024
                ns = 8 if half == 0 else NS
                # strips[i] = (tile, base): tile column = global col - base.
                st = {"strips": [None] * ns, "avs": None}
                db = 8 * half            # first diagonal strip index

                if a < 3:
                    qt, kt, pbase = Q2, K2, 32 * a
                else:
                    qt, kt, pbase = Q2x, K2x, 0

                def seg_mm(ps, si_, c0, c1, po):
                    tt0 = P * si_
                    mm(ps[:, po:po + (c1 - c0)],
                       kt[pbase:pbase + 32, g, :, tt0:tt0 + P],
                       qt[pbase:pbase + 32, g, :, c0:c1],
                       perf_mode=DR, start=True, stop=True)

                def do_scores(i):
                    t0 = P * i
                    s0 = max(t0, tlo)
                    ib = i - db
                    if 5 <= ib <= 7:
                        return         # computed by its pair lead below
                    strip = strip_pool.tile([P, 1024], bf16,
                                            name="strip", tag="strip")
                    ps = sc_ps.tile([P, 1024], f32, name="sc_ps", tag="sc")
                    if 1 <= ib <= 3:
                        # diagonal strips ib and 8-ib have widths summing to
                        # exactly 1024: pack both into one psum tile and one
                        # exp instruction (halves ACT's per-inst overhead)
                        ip = db + 8 - ib
                        t0p = P * ip
                        wA = 1024 - P * ib
                        st["strips"][i] = (strip, s0)
                        st["strips"][ip] = (strip, t0p - wA)
                        seg_mm(ps, i, s0, s0 + CH, 0)
                        seg_mm(ps, i, s0 + CH, s0 + wA, CH)
                        seg_mm(ps, ip, t0p, tlo + 1024, wA)
                        nc.scalar.activation(
                            strip, ps, Exp,
                            scale=float(1.0 / np.sqrt(D) / 4096))
                        return
                    st["strips"][i] = (strip, tlo)
                    b0 = s0
                    while b0 < tlo + 1024:
                        b1 = min((b0 // CH + 1) * CH, tlo + 1024)
                        seg_mm(ps, i, b0, b1, b0 - tlo)
                        b0 = b1
                    nc.scalar.activation(
                        strip[:, s0 - tlo:1024], ps[:, s0 - tlo:1024],
                        Exp, scale=float(1.0 / np.sqrt(D) / 4096))

                def do_av(i):
                    if st["avs"] is None:
                        st["avs"] = {j: av_ps.tile([D + 1, CH], f32,
                                                   name=f"av{j}", tag="av")
                                     for j in (2 * half, 2 * half + 1)}
                    avs = st["avs"]
                    t0 = P * i
                    strip, sb = st["strips"][i]
                    if t0 >= tlo:              # diagonal block: causal mask
                        nc.vector.tensor_mul(
                            strip[:, t0 - sb:t0 + P - sb],
                            strip[:, t0 - sb:t0 + P - sb],
                            tri_sb[:, P:2 * P])
                    for j in (2 * half, 2 * half + 1):
                        if CH * (j + 1) <= t0:
                            continue
                        ts0 = max(CH * j, t0)
                        mm(avs[j][:, ts0 - CH * j:CH],
                           Vsb[:, i, h, :],
                           strip[:, ts0 - sb:CH * (j + 1) - sb],
                           start=(i == 0), stop=(i == 4 * j + 3),
                           skip_group_check=True)
                    if i % 4 == 3 and i // 4 in avs:
                        j = i // 4
                        # a single copy frees the scarce AV psum slot right
                        # away; the rest of the normalize runs off-psum, and
                        # the multiply can then read the rank-1 broadcast
                        # PSUM directly (only one PSUM operand per DVE op)
                        avc = small.tile([D + 1, CH], bf16, name="avc",
                                         tag="avc")
                        nc.vector.tensor_copy(avc, avs[j])
                        rec = small.tile([D + 1, CH], bf16, name="rec", tag="rec")
                        nc.vector.reciprocal(rec[D:D + 1, :], avc[D:D + 1, :])
                        rps = sm_ps.tile([P, CH], f32, name="rps", tag="sm")
                        mm(rps[0:D, :], ones_sb[D:D + 1, 0:D], rec[D:D + 1, :],
                           start=True, stop=True)
                        if h % 2 == 0:
                            # even head: rows land at partitions 0..63 directly
                            nc.vector.tensor_mul(
                                Ysb[h // 2][0:D, CH * j:CH * (j + 1)],
                                avc[0:D, :], rps[0:D, :])
                        else:
                            # odd head: normalize into tmp, then a matmul
                            # against shifti moves the rows to partitions
                            # 64..127 (DVE cannot cross partitions)
                            tmpc = tmp_pool.tile([D, CH], bf16,
                                                 name="tmpc", tag="tmpc")
                            nc.vector.tensor_mul(tmpc, avc[0:D, :],
                                                 rps[0:D, :])
                            shp = sm_ps.tile([P, CH], f32, name="shp", tag="sm")
                            mm(shp, shifti_sb, tmpc, start=True, stop=True)
                            nc.vector.tensor_copy(
                                Ysb[h // 2][D:P, CH * j:CH * (j + 1)],
                                shp[D:P, :])

                return ([lambda i=i: do_scores(i) for i in range(ns)],
                        [lambda i=i: do_av(i) for i in range(ns)])

            # ---------------- global schedule ----------------
            # entries: ("s", scores_fn) participates in the LAG pipeline;
            #          ("f", filler_fn) is PE filler / DMA issue.
            entries = []
            avq = []

            def weave(pass_list, fillers, per, start_after=0):
                """Interleave `fillers` into the passes, `per` fillers after
                each scores entry (fractional via accumulator). The first
                filler is held back until `start_after` scores entries."""
                fi = -per * start_after
                fidx = 0
                for sc, ac in pass_list:
                    for s_fn, a_fn in zip(sc, ac):
                        entries.append(("s", s_fn))
                        avq.append(a_fn)
                        fi += per
                        while fidx + 1 <= fi and fidx < len(fillers):
                            entries.append(("f", fillers[fidx]))
                            fidx += 1
                while fidx < len(fillers):
                    entries.append(("f", fillers[fidx]))
                    fidx += 1

            # setup DMAs in priority order, spread across engine DGE queues:
            # the first projection units need x ch0 + wq/wk only.
            xr0_later = dma_x(0, nc.scalar, split=True)
            dma_w(0, nc.sync)
            xr0_later()
            dma_w(1, nc.gpsimd)
            dma_x(1, nc.sync)
            dma_w(2, nc.scalar)           # wv: needed from V s0 (~unit 9)
            nc.sync.dma_start(
                out=Vsb[:, :, :, D],
                in_=ones[:, 0:NS * HL].rearrange("p (s h) -> p s h", s=NS))
            nc.scalar.dma_start(out=tri_sb, in_=tri)
            nc.scalar.dma_start(out=ones_sb, in_=ones[:, 0:D])
            nc.gpsimd.dma_start(out=shifti_sb, in_=shifti)
            for j in range(NM):
                nc.gpsimd.dma_start(out=Wp_sb[j], in_=wps[j * P:(j + 1) * P, :])

            # minimal prologue: Q/K m0/m1 (heads 0-3) for chunks 0,1 plus the
            # first two V tiles; everything else weaves into the passes.
            pro = []
            for ch in (0, 1):
                for m in (0, 1):
                    pro.append(pu_qk(0, m, ch))
                    pro.append(pu_qk(1, m, ch))
            for ch in (0, 1):
                pro.append(pu_remap(0, 0, ch))
                pro.append(pu_remap(1, 0, ch))
            pro += [pu_v(0), pu_v(1)]
            for f in pro:
                entries.append(("f", f))

            passes0 = [make_pass(h, 0) for h in range(HL)]
            g1_order = [1, 3, 5, 7, 0, 2, 4, 6]
            passes1 = [make_pass(h, 1) for h in g1_order]

            # G0a: first half-0 pass, densely packed with the remaining V
            # tiles its AVs need (V s-tile i is read at stream index i+LAG)
            weave(passes0[:1], [pu_v(s) for s in range(2, 8)], per=6 / 8.0)

            # G0b: remaining half-0 passes; fillers = Q/K m2/m3 (heads 4-7,
            # needed by pass h4 at scores index 24 of this group), chunk-2
            # projections, x2/x3 DMAs
            g0_fill = []
            for m in (2, 3):
                for ch in (0, 1):
                    g0_fill.append(pu_qk(0, m, ch))
                    g0_fill.append(pu_qk(1, m, ch))
            for ch in (0, 1):
                g0_fill.append(pu_remap(0, 1, ch))
                g0_fill.append(pu_remap(1, 1, ch))
            g0_fill.append(lambda: dma_x(2))
            for m in range(NM):
                g0_fill.append(pu_qk(0, m, 2))
                g0_fill.append(pu_qk(1, m, 2))
                if m % 2 == 1:
                    g0_fill.append(pu_remap(0, m // 2, 2))
                    g0_fill.append(pu_remap(1, m // 2, 2))
            g0_fill.append(lambda: dma_x(3))
            # chunk-3 Q/K must complete before any half-1 scores (their
            # first segments read ch3 columns); weaving them as the final
            # G0b fillers preserves that while keeping ACT fed
            for m in range(NM):
                g0_fill.append(pu_qk(0, m, 3))
                g0_fill.append(pu_qk(1, m, 3))
                if m % 2 == 1:
                    g0_fill.append(pu_remap(0, m // 2, 3))
                    g0_fill.append(pu_remap(1, m // 2, 3))
            weave(passes0[1:], g0_fill, per=len(g0_fill) / 56.0)
            # G1a: first half-1 pass densely packed with V s8..15 (V
            # s-tile i is first read by AV(i) at stream index i+LAG, so a
            # 1-per-scores weave lands each tile well before its reader)
            weave(passes1[:1], [pu_v(s) for s in range(8, 16)], per=1.0)

            # G1b: remaining half-1 passes; fillers = chunk-0/1 output
            # projection (ready once all half-0 passes have fully drained
            # through the LAG pipeline - hold fillers back past that point)
            g1_fill = [op_unit(ct, 0) for ct in range(NK)]
            g1_fill += [op_unit(ct, 1) for ct in range(NK)]
            weave(passes1[1:], g1_fill, per=len(g1_fill) / 64.0,
                  start_after=6)

            # tail: chunk-2 output projection can start as soon as the last
            # pass's strip-11 AV (chunk-2 normalize, avq index 187) drains;
            # chunk-3 needs the full drain.
            inject = {64 + 7 * 16 + 11: [op_unit(ct, 2) for ct in range(NK)]}
            tail = [op_unit(ct, 3) for ct in range(NK)]

            # ---------------- execute the stream ----------------
            si = 0
            for kind, fn in entries:
                if kind == "f":
                    fn()
                    continue
                fn()
                if si >= LAG:
                    avq[si - LAG]()
                si += 1
            for k in range(max(si - LAG, 0), si):
                avq[k]()
                for fn in inject.get(k, ()):
                    fn()
            for fn in tail:
                fn()

    nc.compile()
    _nc_cache = nc
    return nc


def _split_qk(Warr):
    """[8, C, 64] -> [C, 512] with column m' = 128*(2g+u) + 32a + dl for
    head h=4g+a, d=32u+dl (matches the Q2/K2 on-device layout)."""
    arr = Warr.reshape(2, 4, C, 2, 32)        # [g, a, c, u, dl]
    arr = arr.transpose(2, 0, 3, 1, 4)        # [c, g, u, a, dl]
    return np.ascontiguousarray(arr.reshape(C, HL * D))


F8 = ml_dtypes.float8_e4m3


def _q8(v):
    return np.asarray(v, F8)


def _pair_layout(arr):
    """[C, m] -> [P, NK/2, 2, m] (contraction split into DoubleRow pairs)."""
    m = arr.shape[1]
    return np.ascontiguousarray(
        arr.reshape(NK // 2, 2, P, m).transpose(2, 0, 1, 3))


def _split8(arr):
    """fp8 value+residual pair of [C, m] array, in pair layout."""
    a8 = _q8(arr)
    r8 = _q8(arr - a8.astype(np.float32))
    return _pair_layout(a8), _pair_layout(r8)


def make_in_maps(x, Wq, Wk, Wv, Wp):
    """Shard FULL inputs into per-core input maps."""
    bf = ml_dtypes.bfloat16
    tri = np.concatenate(
        [np.zeros((P, P), dtype=np.float32),
         np.triu(np.ones((P, P), dtype=np.float32))], axis=1).astype(bf)
    ones = np.ones((P, 2 * P), dtype=np.float32).astype(bf)
    shifti = np.zeros((D, P), dtype=np.float32)
    shifti[np.arange(D), D + np.arange(D)] = 1.0
    shifti = shifti.astype(bf)
    in_maps = []
    for c in range(N_CORES):
        b, g = c // 2, c % 2
        hs = slice(g * HL, (g + 1) * HL)
        x8, xr8 = _split8(np.ascontiguousarray(x[b].T))
        ws = [_split_qk(Wq[hs]), _split_qk(Wk[hs]),
              np.ascontiguousarray(
                  Wv[hs].transpose(1, 0, 2).reshape(C, HL * D))]
        m = {"x8": x8, "xr8": xr8,
             "wps": np.ascontiguousarray(
                 Wp[:, g * HL * D:(g + 1) * HL * D].T).astype(bf),
             "tri": tri, "ones": ones, "shifti": shifti}
        for i, w in enumerate(ws):
            m[f"w8_{i}"], m[f"wr8_{i}"] = _split8(w * 64.0)
        in_maps.append(m)
    return in_maps


def assemble(results, bp):
    """Sum head-group partials per batch, add bias, transpose back."""
    out = np.empty((B, T, C), dtype=np.float32)
    for b in range(B):
        acc = (np.asarray(results[2 * b]["o"], dtype=np.float32)
               + np.asarray(results[2 * b + 1]["o"], dtype=np.float32))
        out[b] = acc.T + bp[None, :]
    return out


def kernel(x, Wq, Wk, Wv, Wp, bp):
    from concourse import bass_utils
    x = np.asarray(x, dtype=np.float32)
    nc = build_nc()
    in_maps = make_in_maps(np.asarray(x), np.asarray(Wq), np.asarray(Wk),
                           np.asarray(Wv), np.asarray(Wp))
    res = bass_utils.run_bass_kernel_spmd(nc, in_maps, core_ids=list(range(N_CORES)))
    return assemble(res.results, np.asarray(bp))


# revision 79
# speedup vs baseline: 1.1230x; 1.0290x over previous
"""Multi-head causal attention (B=4, T=2048, C=1024, H=16, D=64) on 8 trn2 cores.

Sharding: core c owns batch b = c//2 and heads g*8..g*8+7 where g = c%2
(batch-parallel x head-tensor-parallel). Each core computes its 8 heads'
QKV projections, causal attention, and a partial output projection
(columns of Wp belonging to its heads). Host sums the two head-group
partials per batch and adds the bias.

Dtypes (PSUM accumulation always f32):
 - QKV projections: fp8e4m3 value+residual splits of x and 64*W, prepared
   on the host. Three DoubleRow chains (x8*W8 + x8*Wr + xr8*W8) give
   near-bf16 accuracy at 0.5 cycles/row - 25% fewer PE cycles than bf16
   and 2x fewer than f32.
 - scores: fp8e4m3 Q/K with DoubleRow. Q/K are evacuated from the
   projection PSUM straight into tiles laid out [32 partitions, d-half, T]
   per head: the host orders Wq/Wk columns so head a's d-low rows land on
   partitions 32a..32a+31, making the evacuation partition-preserving.
   Matmul APs may only start at partitions 0/32/64, so the a=3 heads
   (partitions 96..127) get a small DMA shift into base-0 copies.
 - everything else (exp strips, V, AV, Y, Wp, o): bf16.
Measured end-to-end error vs the f32 reference ~8.6e-3 (tolerance 2e-2).

Scheduling: one global PE instruction stream, ordered so the ACT engine
(exp - the #2 engine at ~165us busy vs PE ~186us) starts by ~12us and is
never starved mid-stream. Projection and output-projection work is chopped
into ~1-2us units and woven between attention strip-passes as PE filler.
AV matmuls trail scores/exp by LAG strips (software pipeline); per-chunk
softmax normalization (1/rowsum broadcast via a rank-1 PE outer product)
runs as soon as a chunk's last strip lands. Normalized head outputs stay
in SBUF: even heads' rows land on partitions 0..63 directly; odd heads are
moved to partitions 64..127 with a matmul against a shifted identity
(DVE cannot cross partitions). The output projection reads Y from SBUF,
is woven into the second half-passes per chunk, and DMAs out o^T slices
as they complete.

Matmul shapes (out = lhsT.T @ rhs, contraction on partitions):
  QT/KT:  lhsT = W8[ckpair, 2, m-tile] rhs = x8[ckpair, 2, t-chunk] (DR)
  V:      lhsT = x8[ckpair, 2, s-tile] rhs = Wv8[ckpair, 2, :]      (DR)
  scoresT: lhsT = K2[32, 2, s-tile]    rhs = Q2[32, 2, t-seg]       (DR)
  AV^T:   lhsT = [V_h | 1][s-tile,65]  rhs = expT strip [s-tile, t]
  out^T:  lhsT = WpS[j-tile, c-tile]   rhs = Y[j-tile, t-chunk]
"""

import numpy as np
import ml_dtypes
from contextlib import ExitStack

B, T, C, H, D = 4, 2048, 1024, 16, 64
HL = H // 2          # 8 heads per core
N_CORES = 8
P = 128
NK = C // P          # 8 contraction tiles for projections
NM = HL * D // P     # 4 m-tiles of Q/K head-dims
NS = T // P          # 16 s-tiles (key strips)
CH = 512             # t-chunk width
NCH = T // CH        # 4 t-chunks
LAG = 30             # AV trails scores by this many strips

_nc_cache = None


def build_nc():
    global _nc_cache
    if _nc_cache is not None:
        return _nc_cache
    import concourse.bass as bass  # noqa: F401
    import concourse.tile as tile
    from concourse import bacc, mybir

    f32 = mybir.dt.float32
    bf16 = mybir.dt.bfloat16
    fp8 = mybir.dt.float8e4
    Exp = mybir.ActivationFunctionType.Exp
    DR = mybir.MatmulPerfMode.DoubleRow

    def mm(out, lhsT, rhs, **kw):
        nc.tensor.matmul(out, lhsT=lhsT, rhs=rhs, **kw)

    nc = bacc.Bacc("TRN2", target_bir_lowering=False, debug=False,
                   enable_asserts=True, num_devices=N_CORES)
    # x and the QKV weights ship as fp8 value+residual pairs, partition-
    # major: x as [P, NK/2, 2(k-pair), T], W as [P, NK/2, 2, 512], weights
    # prescaled by 64 (psum then holds 64*Q etc.)
    NKP = NK // 2
    x8 = nc.dram_tensor("x8", (P, NKP, 2, T), fp8, kind="ExternalInput").ap()
    xr8 = nc.dram_tensor("xr8", (P, NKP, 2, T), fp8, kind="ExternalInput").ap()
    w8 = [nc.dram_tensor(f"w8_{i}", (P, NKP, 2, HL * D), fp8,
                         kind="ExternalInput").ap() for i in range(3)]
    wr8 = [nc.dram_tensor(f"wr8_{i}", (P, NKP, 2, HL * D), fp8,
                          kind="ExternalInput").ap() for i in range(3)]
    wps = nc.dram_tensor("wps", (HL * D, C), bf16, kind="ExternalInput").ap()
    tri = nc.dram_tensor("tri", (P, 2 * P), bf16, kind="ExternalInput").ap()
    ones = nc.dram_tensor("ones", (P, 2 * P), bf16, kind="ExternalInput").ap()
    # shifti[k, 64+k] = 1: a matmul against it moves rows down 64 partitions
    shifti = nc.dram_tensor("shifti", (D, P), bf16, kind="ExternalInput").ap()
    o = nc.dram_tensor("o", (C, T), bf16, kind="ExternalOutput").ap()

    with tile.TileContext(nc) as tc:
        with ExitStack() as ctx:
            ctx.enter_context(nc.allow_low_precision(
                reason="bf16/fp8 operands with f32 PSUM accumulation"))
            # PSUM banks (8 total): scores 2x[128,1024]=4, small-mm
            # (proj/outproj/rank1/shift) 2x[128,512]=2, av 2x[65,512]=2.
            # Separate tags so filler units never block the scores->exp
            # pipeline's psum rotation.
            sc_ps = ctx.enter_context(tc.tile_pool(name="sc_ps", bufs=2, space="PSUM"))
            sm_ps = ctx.enter_context(tc.tile_pool(name="sm_ps", bufs=2, space="PSUM"))
            av_ps = ctx.enter_context(tc.tile_pool(name="av_ps", bufs=2, space="PSUM"))

            const_pool = ctx.enter_context(tc.tile_pool(name="const", bufs=1))
            tri_sb = const_pool.tile([P, 2 * P], bf16, name="tri_sb", tag="tri_sb")
            ones_sb = const_pool.tile([P, D], bf16, name="ones_sb", tag="ones_sb")
            shifti_sb = const_pool.tile([D, P], bf16, name="shifti_sb",
                                        tag="shifti_sb")

            main = ctx.enter_context(tc.tile_pool(name="main", bufs=1))
            # Q2/K2: [partition 32a+dl, g, u(d-half), t] fp8 - head h=4g+a's
            # (d = 32u+dl) value lives at partition 32a+dl, free (g, u, t).
            # Matmul APs may only start at partitions 0/32/64, so the a=3
            # heads (partitions 96..127) are DMA-shifted into base-0 copies.
            Q2 = main.tile([P, 2, 2, T], fp8, name="q2", tag="q2")
            K2 = main.tile([P, 2, 2, T], fp8, name="k2", tag="k2")
            Q2x = main.tile([32, 2, 2, T], fp8, name="q2x", tag="q2x")
            K2x = main.tile([32, 2, 2, T], fp8, name="k2x", tag="k2x")
            # V: [s-within-tile, s-tile, head, d+1]; col 64 = ones (rowsum)
            Vsb = main.tile([P, NS, HL, D + 1], bf16, name="vsb", tag="vsb")
            Wp_sb = [main.tile([P, C], bf16, name=f"wp{j}", tag=f"wp{j}")
                     for j in range(NM)]
            # normalized head outputs YT, SBUF-resident: tile m holds heads
            # 2m (partitions 0..63) and 2m+1 (64..127)
            Ysb = [main.tile([P, T], bf16, name=f"y{m}", tag=f"y{m}")
                   for m in range(NM)]

            wpool = ctx.enter_context(tc.tile_pool(name="wpool", bufs=1))
            # one consolidated DMA per weight tensor: [p, k-pair, 2, m]
            W_sb = [wpool.tile([P, NKP, 2, HL * D], fp8, name=f"w8{pj}",
                               tag=f"w8{pj}") for pj in range(3)]
            Wr_sb = [wpool.tile([P, NKP, 2, HL * D], fp8, name=f"wr{pj}",
                                tag=f"wr{pj}") for pj in range(3)]

            def dma_w(pj, eng):
                eng.dma_start(out=W_sb[pj], in_=w8[pj])
                eng.dma_start(out=Wr_sb[pj], in_=wr8[pj])

            xpool = ctx.enter_context(tc.tile_pool(name="xpool", bufs=2))
            xs_tiles = {}

            def dma_x(ch, eng=None, split=False):
                xs = xpool.tile([P, NKP, 2, CH], fp8, name="xs", tag="xs")
                xr = xpool.tile([P, NKP, 2, CH], fp8, name="xr", tag="xr")
                e = eng or nc.scalar
                e.dma_start(out=xs, in_=x8[:, :, :, ch * CH:(ch + 1) * CH])
                if not split:
                    e.dma_start(out=xr, in_=xr8[:, :, :, ch * CH:(ch + 1) * CH])
                    xs_tiles[ch] = (xs, xr)
                    return
                xs_tiles[ch] = (xs, xr)

                def later():
                    e.dma_start(out=xr, in_=xr8[:, :, :, ch * CH:(ch + 1) * CH])
                return later

            strip_pool = ctx.enter_context(tc.tile_pool(name="strip_pool", bufs=34))
            small = ctx.enter_context(tc.tile_pool(name="small", bufs=4))
            tmp_pool = ctx.enter_context(tc.tile_pool(name="tmp_pool", bufs=4))
            obpool = ctx.enter_context(tc.tile_pool(name="obpool", bufs=10))

            # ---------------- work units ----------------
            def pu_qk(proj, m, ch):
                """Q or K projection unit -> fp8 split tiles. m = 2g+u.
                Split-fp8: psum = x8*W8 + x8*Wr + xr8*W8 ~= 64*Q exactly."""
                def run():
                    xs, xr = xs_tiles[ch]
                    dst = Q2 if proj == 0 else K2
                    g, u = m // 2, m % 2
                    ms = slice(m * P, (m + 1) * P)
                    ps = sm_ps.tile([P, CH], f32, name="qk_ps", tag="sm")
                    first = True
                    for wt, xt in ((W_sb[proj], xs), (Wr_sb[proj], xs),
                                   (W_sb[proj], xr)):
                        for kp in range(NKP):
                            mm(ps, wt[:, kp, :, ms], xt[:, kp, :, :],
                               perf_mode=DR, start=first,
                               stop=(not first and wt is W_sb[proj]
                                     and xt is xr and kp == NKP - 1))
                            first = False
                    dslice = dst[:, g, u, ch * CH:(ch + 1) * CH]
                    if ch == 2:
                        nc.vector.tensor_copy(dslice, ps)
                    else:
                        nc.scalar.copy(dslice, ps)
                return run

            def pu_remap(proj, g, ch):
                """Shift head a=3's (g, ch) slice down to the base-0 copy."""
                def run():
                    src, dst = (Q2, Q2x) if proj == 0 else (K2, K2x)
                    nc.sync.dma_start(
                        out=dst[:, g, :, ch * CH:(ch + 1) * CH],
                        in_=src[96:128, g, :, ch * CH:(ch + 1) * CH])
                return run

            def pu_v(s):
                """V projection unit for s-tile s (psum = 64*V; evac /64)."""
                def run():
                    ch, sl = s // (CH // P), s % (CH // P)
                    xs, xr = xs_tiles[ch]
                    ss = slice(sl * P, (sl + 1) * P)
                    ps = sm_ps.tile([P, HL * D], f32, name="v_ps", tag="sm")
                    first = True
                    chains = ((xs, W_sb[2]), (xs, Wr_sb[2]), (xr, W_sb[2]))
                    for ci, (xt, wt) in enumerate(chains):
                        for kp in range(NKP):
                            mm(ps, xt[:, kp, :, ss], wt[:, kp, :, :],
                               perf_mode=DR, start=first,
                               stop=(ci == 2 and kp == NKP - 1))
                            first = False
                    # evac on ACT where it idles (pre-pass region);
                    # on DVE in G1 where ACT is the exp bottleneck. The
                    # 1/64 de-scale fuses into either copy.
                    if s < 8:
                        nc.scalar.activation(
                            Vsb[:, s, :, 0:D],
                            ps.rearrange("p (h d) -> p h d", h=HL),
                            mybir.ActivationFunctionType.Copy, scale=1.0 / 64)
                    else:
                        nc.vector.tensor_scalar_mul(
                            Vsb[:, s, :, 0:D],
                            ps.rearrange("p (h d) -> p h d", h=HL), 1.0 / 64)
                return run

            def op_unit(ct, ch):
                """Output-projection unit: o^T[ct-tile, ch-chunk]."""
                def run():
                    ps = sm_ps.tile([P, CH], f32, name="p_ps", tag="sm")
                    for j in range(NM):
                        mm(ps, Wp_sb[j][:, ct * P:(ct + 1) * P],
                           Ysb[j][:, ch * CH:(ch + 1) * CH],
                           start=(j == 0), stop=(j == NM - 1))
                    ob = obpool.tile([P, CH], bf16, name="ob", tag="ob")
                    if ch >= 2:
                        # tail chunks run after the last exp - ACT is free
                        nc.scalar.copy(ob, ps)
                    else:
                        nc.vector.tensor_copy(ob, ps)
                    eng = (nc.sync, nc.gpsimd)[(ct + ch) % 2]
                    eng.dma_start(
                        out=o[ct * P:(ct + 1) * P, ch * CH:(ch + 1) * CH], in_=ob)
                return run

            # ---------------- attention passes ----------------
            def make_pass(h, half):
                g, a = h // 4, h % 4
                tlo = half * 1024
                ns = 8 if half == 0 else NS
                # strips[i] = (tile, base): tile column = global col - base.
                st = {"strips": [None] * ns, "avs": None}
                db = 8 * half            # first diagonal strip index

                if a < 3:
                    qt, kt, pbase = Q2, K2, 32 * a
                else:
                    qt, kt, pbase = Q2x, K2x, 0

                def seg_mm(ps, si_, c0, c1, po):
                    tt0 = P * si_
                    mm(ps[:, po:po + (c1 - c0)],
                       kt[pbase:pbase + 32, g, :, tt0:tt0 + P],
                       qt[pbase:pbase + 32, g, :, c0:c1],
                       perf_mode=DR, start=True, stop=True)

                def do_scores(i):
                    t0 = P * i
                    s0 = max(t0, tlo)
                    ib = i - db
                    if 5 <= ib <= 7:
                        return         # computed by its pair lead below
                    strip = strip_pool.tile([P, 1024], bf16,
                                            name="strip", tag="strip")
                    ps = sc_ps.tile([P, 1024], f32, name="sc_ps", tag="sc")
                    if 1 <= ib <= 3:
                        # diagonal strips ib and 8-ib have widths summing to
                        # exactly 1024: pack both into one psum tile and one
                        # exp instruction (halves ACT's per-inst overhead)
                        ip = db + 8 - ib
                        t0p = P * ip
                        wA = 1024 - P * ib
                        st["strips"][i] = (strip, s0)
                        st["strips"][ip] = (strip, t0p - wA)
                        seg_mm(ps, i, s0, s0 + CH, 0)
                        seg_mm(ps, i, s0 + CH, s0 + wA, CH)
                        seg_mm(ps, ip, t0p, tlo + 1024, wA)
                        nc.scalar.activation(
                            strip, ps, Exp,
                            scale=float(1.0 / np.sqrt(D) / 4096))
                        return
                    st["strips"][i] = (strip, tlo)
                    b0 = s0
                    while b0 < tlo + 1024:
                        b1 = min((b0 // CH + 1) * CH, tlo + 1024)
                        seg_mm(ps, i, b0, b1, b0 - tlo)
                        b0 = b1
                    nc.scalar.activation(
                        strip[:, s0 - tlo:1024], ps[:, s0 - tlo:1024],
                        Exp, scale=float(1.0 / np.sqrt(D) / 4096))

                def do_av(i):
                    if st["avs"] is None:
                        st["avs"] = {j: av_ps.tile([D + 1, CH], f32,
                                                   name=f"av{j}", tag="av")
                                     for j in (2 * half, 2 * half + 1)}
                    avs = st["avs"]
                    t0 = P * i
                    strip, sb = st["strips"][i]
                    if t0 >= tlo:              # diagonal block: causal mask
                        nc.vector.tensor_mul(
                            strip[:, t0 - sb:t0 + P - sb],
                            strip[:, t0 - sb:t0 + P - sb],
                            tri_sb[:, P:2 * P])
                    for j in (2 * half, 2 * half + 1):
                        if CH * (j + 1) <= t0:
                            continue
                        ts0 = max(CH * j, t0)
                        mm(avs[j][:, ts0 - CH * j:CH],
                           Vsb[:, i, h, :],
                           strip[:, ts0 - sb:CH * (j + 1) - sb],
                           start=(i == 0), stop=(i == 4 * j + 3),
                           skip_group_check=True)
                    if i % 4 == 3 and i // 4 in avs:
                        j = i // 4
                        # a single copy frees the scarce AV psum slot right
                        # away; the rest of the normalize runs off-psum, and
                        # the multiply can then read the rank-1 broadcast
                        # PSUM directly (only one PSUM operand per DVE op)
                        avc = small.tile([D + 1, CH], bf16, name="avc",
                                         tag="avc")
                        nc.vector.tensor_copy(avc, avs[j])
                        rec = small.tile([D + 1, CH], bf16, name="rec", tag="rec")
                        nc.vector.reciprocal(rec[D:D + 1, :], avc[D:D + 1, :])
                        rps = sm_ps.tile([P, CH], f32, name="rps", tag="sm")
                        mm(rps[0:D, :], ones_sb[D:D + 1, 0:D], rec[D:D + 1, :],
                           start=True, stop=True)
                        if h % 2 == 0:
                            # even head: rows land at partitions 0..63 directly
                            nc.vector.tensor_mul(
                                Ysb[h // 2][0:D, CH * j:CH * (j + 1)],
                                avc[0:D, :], rps[0:D, :])
                        else:
                            # odd head: normalize into tmp, then a matmul
                            # against shifti moves the rows to partitions
                            # 64..127 (DVE cannot cross partitions)
                            tmpc = tmp_pool.tile([D, CH], bf16,
                                                 name="tmpc", tag="tmpc")
                            nc.vector.tensor_mul(tmpc, avc[0:D, :],
                                                 rps[0:D, :])
                            shp = sm_ps.tile([P, CH], f32, name="shp", tag="sm")
                            mm(shp, shifti_sb, tmpc, start=True, stop=True)
                            nc.vector.tensor_copy(
                                Ysb[h // 2][D:P, CH * j:CH * (j + 1)],
                                shp[D:P, :])

                return ([lambda i=i: do_scores(i) for i in range(ns)],
                        [lambda i=i: do_av(i) for i in range(ns)])

            # ---------------- global schedule ----------------
            # entries: ("s", scores_fn) participates in the LAG pipeline;
            #          ("f", filler_fn) is PE filler / DMA issue.
            entries = []
            avq = []

            def weave(pass_list, fillers, per, start_after=0):
                """Interleave `fillers` into the passes, `per` fillers after
                each scores entry (fractional via accumulator). The first
                filler is held back until `start_after` scores entries."""
                fi = -per * start_after
                fidx = 0
                for sc, ac in pass_list:
                    for s_fn, a_fn in zip(sc, ac):
                        entries.append(("s", s_fn))
                        avq.append(a_fn)
                        fi += per
                        while fidx + 1 <= fi and fidx < len(fillers):
                            entries.append(("f", fillers[fidx]))
                            fidx += 1
                while fidx < len(fillers):
                    entries.append(("f", fillers[fidx]))
                    fidx += 1

            # setup DMAs in priority order, spread across engine DGE queues:
            # the first projection units need x ch0 + wq/wk only.
            xr0_later = dma_x(0, nc.scalar, split=True)
            dma_w(0, nc.sync)
            xr0_later()
            dma_w(1, nc.gpsimd)
            dma_x(1, nc.sync)
            dma_w(2, nc.scalar)           # wv: needed from V s0 (~unit 9)
            nc.sync.dma_start(
                out=Vsb[:, :, :, D],
                in_=ones[:, 0:NS * HL].rearrange("p (s h) -> p s h", s=NS))
            nc.scalar.dma_start(out=tri_sb, in_=tri)
            nc.scalar.dma_start(out=ones_sb, in_=ones[:, 0:D])
            nc.gpsimd.dma_start(out=shifti_sb, in_=shifti)
            for j in range(NM):
                nc.gpsimd.dma_start(out=Wp_sb[j], in_=wps[j * P:(j + 1) * P, :])

            # minimal prologue: Q/K m0/m1 (heads 0-3) for chunks 0,1 plus the
            # first two V tiles; everything else weaves into the passes.
            pro = []
            for ch in (0, 1):
                for m in (0, 1):
                    pro.append(pu_qk(0, m, ch))
                    pro.append(pu_qk(1, m, ch))
            for ch in (0, 1):
                pro.append(pu_remap(0, 0, ch))
                pro.append(pu_remap(1, 0, ch))
            pro += [pu_v(0), pu_v(1)]
            for f in pro:
                entries.append(("f", f))

            passes0 = [make_pass(h, 0) for h in range(HL)]
            g1_order = [1, 3, 5, 7, 0, 2, 4, 6]
            passes1 = [make_pass(h, 1) for h in g1_order]

            # G0a: first half-0 pass, densely packed with the remaining V
            # tiles its AVs need (V s-tile i is read at stream index i+LAG)
            weave(passes0[:1], [pu_v(s) for s in range(2, 8)], per=6 / 8.0)

            # G0b: remaining half-0 passes; fillers = Q/K m2/m3 (heads 4-7,
            # needed by pass h4 at scores index 24 of this group), chunk-2
            # projections, x2/x3 DMAs
            g0_fill = []
            for m in (2, 3):
                for ch in (0, 1):
                    g0_fill.append(pu_qk(0, m, ch))
                    g0_fill.append(pu_qk(1, m, ch))
            for ch in (0, 1):
                g0_fill.append(pu_remap(0, 1, ch))
                g0_fill.append(pu_remap(1, 1, ch))
            g0_fill.append(lambda: dma_x(2))
            for m in range(NM):
                g0_fill.append(pu_qk(0, m, 2))
                g0_fill.append(pu_qk(1, m, 2))
                if m % 2 == 1:
                    g0_fill.append(pu_remap(0, m // 2, 2))
                    g0_fill.append(pu_remap(1, m // 2, 2))
            g0_fill.append(lambda: dma_x(3))
            # chunk-3 Q/K must complete before any half-1 scores (their
            # first segments read ch3 columns); weaving them as the final
            # G0b fillers preserves that while keeping ACT fed
            for m in range(NM):
                g0_fill.append(pu_qk(0, m, 3))
                g0_fill.append(pu_qk(1, m, 3))
                if m % 2 == 1:
                    g0_fill.append(pu_remap(0, m // 2, 3))
                    g0_fill.append(pu_remap(1, m // 2, 3))
            weave(passes0[1:], g0_fill, per=len(g0_fill) / 56.0)
            # G1a: first half-1 pass densely packed with V s8..15 (V
            # s-tile i is first read by AV(i) at stream index i+LAG, so a
            # 1-per-scores weave lands each tile well before its reader)
            weave(passes1[:1], [pu_v(s) for s in range(8, 16)], per=1.0)

            # G1b: remaining half-1 passes; fillers = chunk-0/1 output
            # projection (ready once all half-0 passes have fully drained
            # through the LAG pipeline - hold fillers back past that point)
            g1_fill = [op_unit(ct, 0) for ct in range(NK)]
            g1_fill += [op_unit(ct, 1) for ct in range(NK)]
            weave(passes1[1:], g1_fill, per=len(g1_fill) / 64.0,
                  start_after=6)

            # tail: chunk-2 output projection can start as soon as the last
            # pass's strip-11 AV (chunk-2 normalize, avq index 187) drains;
            # chunk-3 needs the full drain.
            inject = {64 + 7 * 16 + 11: [op_unit(ct, 2) for ct in range(NK)]}
            tail = [op_unit(ct, 3) for ct in range(NK)]

            # ---------------- execute the stream ----------------
            si = 0
            for kind, fn in entries:
                if kind == "f":
                    fn()
                    continue
                fn()
                if si >= LAG:
                    avq[si - LAG]()
                si += 1
            for k in range(max(si - LAG, 0), si):
                avq[k]()
                for fn in inject.get(k, ()):
                    fn()
            for fn in tail:
                fn()

    nc.compile()
    _nc_cache = nc
    return nc


def _split_qk(Warr):
    """[8, C, 64] -> [C, 512] with column m' = 128*(2g+u) + 32a + dl for
    head h=4g+a, d=32u+dl (matches the Q2/K2 on-device layout)."""
    arr = Warr.reshape(2, 4, C, 2, 32)        # [g, a, c, u, dl]
    arr = arr.transpose(2, 0, 3, 1, 4)        # [c, g, u, a, dl]
    return np.ascontiguousarray(arr.reshape(C, HL * D))


F8 = ml_dtypes.float8_e4m3


def _q8(v):
    return np.asarray(v, F8)


def _pair_layout(arr):
    """[C, m] -> [P, NK/2, 2, m] (contraction split into DoubleRow pairs)."""
    m = arr.shape[1]
    return np.ascontiguousarray(
        arr.reshape(NK // 2, 2, P, m).transpose(2, 0, 1, 3))


def _split8(arr):
    """fp8 value+residual pair of [C, m] array, in pair layout."""
    a8 = _q8(arr)
    r8 = _q8(arr - a8.astype(np.float32))
    return _pair_layout(a8), _pair_layout(r8)


def make_in_maps(x, Wq, Wk, Wv, Wp):
    """Shard FULL inputs into per-core input maps."""
    bf = ml_dtypes.bfloat16
    tri = np.concatenate(
        [np.zeros((P, P), dtype=np.float32),
         np.triu(np.ones((P, P), dtype=np.float32))], axis=1).astype(bf)
    ones = np.ones((P, 2 * P), dtype=np.float32).astype(bf)
    shifti = np.zeros((D, P), dtype=np.float32)
    shifti[np.arange(D), D + np.arange(D)] = 1.0
    shifti = shifti.astype(bf)
    in_maps = []
    for c in range(N_CORES):
        b, g = c // 2, c % 2
        hs = slice(g * HL, (g + 1) * HL)
        x8, xr8 = _split8(np.ascontiguousarray(x[b].T))
        ws = [_split_qk(Wq[hs]), _split_qk(Wk[hs]),
              np.ascontiguousarray(
                  Wv[hs].transpose(1, 0, 2).reshape(C, HL * D))]
        m = {"x8": x8, "xr8": xr8,
             "wps": np.ascontiguousarray(
                 Wp[:, g * HL * D:(g + 1) * HL * D].T).astype(bf),
             "tri": tri, "ones": ones, "shifti": shifti}
        for i, w in enumerate(ws):
            m[f"w8_{i}"], m[f"wr8_{i}"] = _split8(w * 64.0)
        in_maps.append(m)
    return in_maps


def assemble(results, bp):
    """Sum head-group partials per batch, add bias, transpose back."""
    out = np.empty((B, T, C), dtype=np.float32)
    for b in range(B):
        acc = (np.asarray(results[2 * b]["o"], dtype=np.float32)
               + np.asarray(results[2 * b + 1]["o"], dtype=np.float32))
        out[b] = acc.T + bp[None, :]
    return out


def kernel(x, Wq, Wk, Wv, Wp, bp):
    from concourse import bass_utils
    x = np.asarray(x, dtype=np.float32)
    nc = build_nc()
    in_maps = make_in_maps(np.asarray(x), np.asarray(Wq), np.asarray(Wk),
                           np.asarray(Wv), np.asarray(Wp))
    res = bass_utils.run_bass_kernel_spmd(nc, in_maps, core_ids=list(range(N_CORES)))
    return assemble(res.results, np.asarray(bp))
